# revision 5
# baseline (speedup 1.0000x reference)
"""Distributed HSIC independence loss for Trainium2 (8 NeuronCores).

Pipeline (single NEFF launch, row-sharded across 8 cores):
  1. Per core: P = Zrow @ Zfull.T via TensorE (bf16, f32 accum), with the
     -|z_j|^2/2 term folded in as two extra bf16 contraction rows (hi+lo
     split), so d2 = -2*P + |z_i|^2 comes out of PSUM in one ScalarE
     activation (stored shifted, fp16).
  2. Median of d2: host supplies a sampled estimate t0; the device computes
     exact full counts of d2 <= t0 +/- h, AllReduces the 4 counts (Z and N),
     and linearly interpolates the CDF to get the global lower-median.
  3. K = exp(-d2/(2*sigma^2+1e-8)) via one ScalarE activation per tile
     (runtime per-partition scale/bias), with fused row-sum accumulation.
  4. Row sums are AllGathered; HSIC sum computed via
     sum(Kc*Lc) = sum((v_j - K)(q_j - L)) - n * sum_i alpha_i*beta_i
     with alpha_i = mu_i - mean, so no per-element centering bias passes.
  5. Per-core partials summed on host; divide by (n-1)^2 + 1e-8.
"""

import numpy as np
import ml_dtypes
from contextlib import ExitStack

NCORES = 8
NTOT = 4096
DZ = 512
DN = 128
BLK = NTOT // NCORES      # 512 rows per core
MT = BLK // 128           # 4 M-tiles per core
NB = NTOT // 512          # 8 column tiles of 512
SH_Z = 1024.0             # fp16 storage shift for d2 of Z
SH_N = 256.0
HZ = 12.0                 # count-threshold half-window
HN = 3.0
KTARGET = float((NTOT * NTOT - 1) // 2 + 1)   # 8388608: lower-median rank

_BF16 = ml_dtypes.bfloat16

_nc_cache = {}


def _split_waits(nc, limit=1):
    """This walrus build accepts at most one sync-wait per instruction;
    hoist extra waits onto preceding single-wait drains on the same engine."""
    import concourse.mybir as mybir
    import bass_rust
    ctr = 0
    for f in nc.m.functions:
        for b in f.blocks:
            out, changed = [], False
            for inst in b.instructions:
                si = inst.sync_info
                waits = list(si.on_wait) if si is not None else []
                if len(waits) > limit:
                    changed = True
                    for w in waits[:-limit]:
                        ctr += 1
                        d = mybir.InstDrain(name=f"I-waitsplit-{ctr}", ins=[], outs=[])
                        d.engine = inst.engine
                        d.sync_info = bass_rust.SyncInfo(on_update=[], on_wait=[w])
                        out.append(d)
                    si.on_wait = waits[-limit:]
                out.append(inst)
            if changed:
                b.instructions = out
    return ctr


def _build():
    import concourse.bass as bass
    import concourse.mybir as mybir
    import concourse.tile as tile
    from concourse import bass_isa

    f32 = mybir.dt.float32
    f16 = mybir.dt.float16
    bf16 = mybir.dt.bfloat16
    Alu = mybir.AluOpType
    Act = mybir.ActivationFunctionType
    RG = [list(range(NCORES))]

    nc = bass.Bass("TRN2", num_devices=NCORES)

    zt = nc.dram_tensor("zt", [DZ + 2, NTOT], bf16, kind="ExternalInput")
    ntr = nc.dram_tensor("ntr", [DN + 2, NTOT], bf16, kind="ExternalInput")
    lhsz = nc.dram_tensor("lhsz", [DZ, BLK], bf16, kind="ExternalInput")
    lhsn = nc.dram_tensor("lhsn", [DN, BLK], bf16, kind="ExternalInput")
    zsqm = nc.dram_tensor("zsqm", [BLK], f32, kind="ExternalInput")   # |z_i|^2 - SH_Z
    nsqm = nc.dram_tensor("nsqm", [BLK], f32, kind="ExternalInput")   # |n_i|^2 - SH_N
    thr = nc.dram_tensor("thr", [4], f32, kind="ExternalInput")       # shifted thresholds
    out_s = nc.dram_tensor("out_s", [1, 1], f32, kind="ExternalOutput")
    out_dbg = nc.dram_tensor("out_dbg", [1, 8], f32, kind="ExternalOutput")

    KZT = DZ // 128   # 4 contraction tiles for Z
    KNT = DN // 128   # 1 for N

    with tile.TileContext(nc) as tc, ExitStack() as ctx:
        big = ctx.enter_context(tc.tile_pool(name="big", bufs=1))
        psum = ctx.enter_context(tc.tile_pool(name="psum", bufs=8, space="PSUM"))
        small = ctx.enter_context(tc.tile_pool(name="small", bufs=1))
        dram = ctx.enter_context(tc.tile_pool(name="dram", bufs=1, space="DRAM"))

        # ---------------- input DMAs ----------------
        zt_sb = []
        for k in range(KZT):
            t = big.tile([128, NTOT], bf16, tag=f"zk{k}", name=f"zt_sb{k}")
            nc.sync.dma_start(t[:], zt[k * 128:(k + 1) * 128, :])
            zt_sb.append(t)
        ztw = small.tile([2, NTOT], bf16, tag="ztw", name="ztw")
        nc.sync.dma_start(ztw[:], zt[DZ:DZ + 2, :])

        nt_sb = big.tile([128, NTOT], bf16, tag="nk0", name="nt_sb")
        nc.sync.dma_start(nt_sb[:], ntr[0:128, :])
        ntw = small.tile([2, NTOT], bf16, tag="ntw", name="ntw")
        nc.sync.dma_start(ntw[:], ntr[DN:DN + 2, :])

        lhsz_sb = []
        for k in range(KZT):
            t = small.tile([128, BLK], bf16, tag=f"lz{k}", name=f"lhsz_sb{k}")
            nc.sync.dma_start(t[:], lhsz[k * 128:(k + 1) * 128, :])
            lhsz_sb.append(t)
        lhsn_sb = small.tile([128, BLK], bf16, tag="ln0", name="lhsn_sb")
        nc.sync.dma_start(lhsn_sb[:], lhsn[:, :])

        ones2 = small.tile([2, 128], bf16, tag="ones2", name="ones2")
        nc.vector.memset(ones2[:], 1.0)

        zsqm_sb = small.tile([128, MT], f32, tag="zsqm", name="zsqm_sb")
        nc.sync.dma_start(zsqm_sb[:], zsqm[:].rearrange("(m p) -> p m", p=128))
        nsqm_sb = small.tile([128, MT], f32, tag="nsqm", name="nsqm_sb")
        nc.sync.dma_start(nsqm_sb[:], nsqm[:].rearrange("(m p) -> p m", p=128))

        thrb = small.tile([128, 4], f32, tag="thrb", name="thrb")
        thr_ap = thr[:]
        thr_b = bass.AP(tensor=thr_ap.tensor, offset=thr_ap.offset,
                        ap=[[0, 128], [1, 4]])
        nc.sync.dma_start(thrb[:], thr_b)

        # ---------------- matmuls + d2s evacuation ----------------
        def mm_phase(d2s_tiles, lhs_tiles, rhs_tiles, wtile, sq_sb, kt, mat):
            for m in range(MT):
                ps = []
                for nb in range(NB):
                    p = psum.tile([128, 512], f32, tag="ps", name=f"ps_{mat}{m}_{nb}")
                    ps.append(p)
                for k in range(kt):
                    lw = lhs_tiles[k][:, m * 128:(m + 1) * 128]
                    for nb in range(NB):
                        nc.tensor.matmul(ps[nb][:], lw,
                                         rhs_tiles[k][:, nb * 512:(nb + 1) * 512],
                                         start=(k == 0), stop=False)
                for nb in range(NB):
                    nc.tensor.matmul(ps[nb][:], ones2[:, 0:128],
                                     wtile[:, nb * 512:(nb + 1) * 512],
                                     start=False, stop=True)
                for nb in range(NB):
                    nc.scalar.activation(d2s_tiles[m][:, nb * 512:(nb + 1) * 512],
                                         ps[nb][:], Act.Identity,
                                         bias=sq_sb[:, m:m + 1], scale=-2.0)

        d2sz = [big.tile([128, NTOT], f16, tag=f"dz{m}", name=f"d2sz{m}")
                for m in range(MT)]
        mm_phase(d2sz, lhsz_sb, zt_sb, ztw, zsqm_sb, KZT, "z")

        d2sn = [big.tile([128, NTOT], f16, tag=f"dn{m}", name=f"d2sn{m}")
                for m in range(MT)]
        mm_phase(d2sn, [lhsn_sb], [nt_sb], ntw, nsqm_sb, KNT, "n")

        # ---------------- exact CDF counts at host thresholds ----------------
        scr16 = big.tile([128, NTOT], f16, tag="scr", name="scr16")
        cntz = small.tile([128, 2, MT], f32, tag="cntz", name="cntz")
        cntn = small.tile([128, 2, MT], f32, tag="cntn", name="cntn")
        for t in range(2):
            for m in range(MT):
                nc.vector.tensor_scalar(scr16[:], d2sz[m][:], thrb[:, t:t + 1], None,
                                        Alu.is_le, Alu.add,
                                        accum_out=cntz[:, t, m:m + 1])
        for t in range(2):
            for m in range(MT):
                nc.vector.tensor_scalar(scr16[:], d2sn[m][:], thrb[:, 2 + t:3 + t], None,
                                        Alu.is_le, Alu.add,
                                        accum_out=cntn[:, t, m:m + 1])

        cc = small.tile([128, 4], f32, tag="cc", name="cc")
        nc.vector.tensor_reduce(cc[:, 0:2], cntz[:], mybir.AxisListType.X, Alu.add)
        nc.vector.tensor_reduce(cc[:, 2:4], cntn[:], mybir.AxisListType.X, Alu.add)

        # partition-sum of the counts via a ones-vector matmul on PE
        ones1 = small.tile([128, 1], f32, tag="ones1", name="ones1")
        nc.vector.memset(ones1[:], 1.0)
        ccp = psum.tile([4, 1], f32, tag="ps", name="ccp")
        nc.tensor.matmul(ccp[:], cc[:], ones1[:], start=True, stop=True)
        ccs = small.tile([4, 1], f32, tag="ccs", name="ccs")
        nc.scalar.activation(ccs[:], ccp[:], Act.Identity)

        cc_in = dram.tile([1, 4], f32, tag="cc_in", name="cc_in")
        cc_out = dram.tile([1, 4], f32, tag="cc_out", name="cc_out")
        cci_ap = cc_in[:]
        nc.sync.dma_start(
            bass.AP(tensor=cci_ap.tensor, offset=cci_ap.offset, ap=[[1, 4], [4, 1]]),
            ccs[:])
        nc.gpsimd.collective_compute("AllReduce", Alu.add, replica_groups=RG,
                                     ins=[cc_in[:]], outs=[cc_out[:]])
        ccg = small.tile([128, 4], f32, tag="ccg", name="ccg")
        cco_ap = cc_out[:]
        nc.sync.dma_start(
            ccg[:],
            bass.AP(tensor=cco_ap.tensor, offset=cco_ap.offset, ap=[[0, 128], [1, 4]]))

        # ---------------- median interpolation + exp coefficients ----------------
        def interp(c0, c1, t0ap, h, shift, mat):
            num = small.tile([128, 1], f32, tag=f"num{mat}", name=f"num{mat}")
            nc.vector.tensor_scalar(num[:], c0, KTARGET, -1.0, Alu.subtract,
                                    Alu.mult)                  # (C0-k)*-1 = k-C0
            den = small.tile([128, 1], f32, tag=f"den{mat}", name=f"den{mat}")
            nc.vector.tensor_sub(den[:], c1, c0)
            rec = small.tile([128, 1], f32, tag=f"rec{mat}", name=f"rec{mat}")
            nc.vector.reciprocal(rec[:], den[:])
            r = small.tile([128, 1], f32, tag=f"r{mat}", name=f"r{mat}")
            nc.vector.tensor_mul(r[:], num[:], rec[:])
            rc = small.tile([128, 1], f32, tag=f"rc{mat}", name=f"rc{mat}")
            nc.vector.tensor_scalar(rc[:], r[:], 0.0, 1.0, Alu.max, Alu.min)
            meds = small.tile([128, 1], f32, tag=f"meds{mat}", name=f"meds{mat}")
            nc.vector.tensor_scalar(meds[:], rc[:], 2.0 * h, t0ap, Alu.mult, Alu.add)
            tmp = small.tile([128, 1], f32, tag=f"tmp{mat}", name=f"tmp{mat}")
            nc.vector.tensor_scalar(tmp[:], meds[:], shift + 3e-8, None, Alu.add)
            s = small.tile([128, 1], f32, tag=f"s{mat}", name=f"s{mat}")
            nc.vector.reciprocal(s[:], tmp[:])
            sc = small.tile([128, 1], f32, tag=f"sc{mat}", name=f"sc{mat}")
            nc.vector.tensor_scalar(sc[:], s[:], -1.0, None, Alu.mult)
            bs = small.tile([128, 1], f32, tag=f"bs{mat}", name=f"bs{mat}")
            nc.vector.tensor_scalar(bs[:], s[:], -shift, None, Alu.mult)
            return meds, sc, bs

        medz, scz, bsz = interp(ccg[:, 0:1], ccg[:, 1:2], thrb[:, 0:1], HZ, SH_Z, "z")
        medn, scn, bsn = interp(ccg[:, 2:3], ccg[:, 3:4], thrb[:, 2:3], HN, SH_N, "n")

        # ---------------- exp + fused row sums ----------------
        kz = [big.tile([128, NTOT], f16, tag=f"zk{m}", name=f"kz{m}")
              for m in range(MT)]
        rz = small.tile([128, MT], f32, tag="rz", name="rz")
        for m in range(MT):
            nc.scalar.activation(kz[m][:], d2sz[m][:], Act.Exp,
                                 bias=bsz[:], scale=scz[:],
                                 accum_out=rz[:, m:m + 1])
        kn = [big.tile([128, NTOT], f16, tag=f"dz{m}", name=f"kn{m}")
              for m in range(MT)]
        rn = small.tile([128, MT], f32, tag="rn", name="rn")
        for m in range(MT):
            nc.scalar.activation(kn[m][:], d2sn[m][:], Act.Exp,
                                 bias=bsn[:], scale=scn[:],
                                 accum_out=rn[:, m:m + 1])

        # ---------------- AllGather row sums ----------------
        ag_in = dram.tile([1, 2 * BLK], f32, tag="ag_in", name="ag_in")
        ag_out = dram.tile([NCORES, 2 * BLK], f32, tag="ag_out", name="ag_out")
        agi_ap = ag_in[:]
        dst_z = bass.AP(tensor=agi_ap.tensor, offset=agi_ap.offset,
                        ap=[[1, 128], [128, MT]])
        nc.sync.dma_start(dst_z, rz[:])
        dst_n = bass.AP(tensor=agi_ap.tensor, offset=agi_ap.offset + BLK,
                        ap=[[1, 128], [128, MT]])
        nc.sync.dma_start(dst_n, rn[:])
        nc.gpsimd.collective_compute("AllGather", Alu.bypass, replica_groups=RG,
                                     ins=[ag_in[:]], outs=[ag_out[:]])

        # broadcast row-sum vectors to all partitions: V[p, j] = R_all[j]
        ago_ap = ag_out[:]
        vz = big.tile([128, NCORES, BLK], f32, tag="vz", name="vz")
        src_z = bass.AP(tensor=ago_ap.tensor, offset=ago_ap.offset,
                        ap=[[0, 128], [2 * BLK, NCORES], [1, BLK]])
        nc.sync.dma_start(vz[:], src_z)
        vn = big.tile([128, NCORES, BLK], f32, tag="vn", name="vn")
        src_n = bass.AP(tensor=ago_ap.tensor, offset=ago_ap.offset + BLK,
                        ap=[[0, 128], [2 * BLK, NCORES], [1, BLK]])
        nc.sync.dma_start(vn[:], src_n)

        # ---------------- means / alpha / correction ----------------
        inv_n = 1.0 / NTOT
        inv_n2 = 1.0 / (NTOT * NTOT)
        tz = small.tile([128, 1], f32, tag="tz", name="tz")
        nc.vector.tensor_reduce(tz[:], vz[:], mybir.AxisListType.XY, Alu.add)
        tn = small.tile([128, 1], f32, tag="tn", name="tn")
        nc.vector.tensor_reduce(tn[:], vn[:], mybir.AxisListType.XY, Alu.add)
        mbz = small.tile([128, 1], f32, tag="mbz", name="mbz")
        nc.vector.tensor_scalar(mbz[:], tz[:], inv_n2, None, Alu.mult)
        mbn = small.tile([128, 1], f32, tag="mbn", name="mbn")
        nc.vector.tensor_scalar(mbn[:], tn[:], inv_n2, None, Alu.mult)
        az = small.tile([128, MT], f32, tag="az", name="az")
        nc.vector.tensor_scalar(az[:], rz[:], inv_n, mbz[:], Alu.mult, Alu.subtract)
        an = small.tile([128, MT], f32, tag="an", name="an")
        nc.vector.tensor_scalar(an[:], rn[:], inv_n, mbn[:], Alu.mult, Alu.subtract)
        ca = small.tile([128, MT], f32, tag="ca", name="ca")
        nc.vector.tensor_mul(ca[:], az[:], an[:])

        # ---------------- final centered product ----------------
        pacc = small.tile([128, MT], f32, tag="pacc", name="pacc")
        vz2 = vz[:].rearrange("p c b -> p (c b)")
        vn2 = vn[:].rearrange("p c b -> p (c b)")
        for m in range(MT):
            a_t = big.tile([128, NTOT], f16, tag="A", name=f"a_t{m}")
            nc.vector.scalar_tensor_tensor(a_t[:], vz2, inv_n, kz[m][:],
                                           Alu.mult, Alu.subtract)
            b_t = big.tile([128, NTOT], f16, tag="B", name=f"b_t{m}")
            nc.vector.scalar_tensor_tensor(b_t[:], vn2, inv_n, kn[m][:],
                                           Alu.mult, Alu.subtract)
            nc.vector.scalar_tensor_tensor(
                scr16[:], a_t[:], 1.0, b_t[:], Alu.mult, Alu.mult,
                accum_out=pacc[:, m:m + 1])

        sa = small.tile([128, 1], f32, tag="sa", name="sa")
        nc.vector.tensor_reduce(sa[:], pacc[:], mybir.AxisListType.X, Alu.add)
        cb = small.tile([128, 1], f32, tag="cb", name="cb")
        nc.vector.tensor_reduce(cb[:], ca[:], mybir.AxisListType.X, Alu.add)
        spp = small.tile([128, 1], f32, tag="spp", name="spp")
        nc.vector.tensor_scalar(spp[:], cb[:], -float(NTOT), sa[:],
                                Alu.mult, Alu.add)
        stp = psum.tile([1, 1], f32, tag="ps", name="stp")
        nc.tensor.matmul(stp[:], spp[:], ones1[:], start=True, stop=True)
        stot = small.tile([1, 1], f32, tag="stot", name="stot")
        nc.scalar.activation(stot[:], stp[:], Act.Identity)
        nc.sync.dma_start(out_s[:], stot[0:1, 0:1])

        # debug outputs
        nc.sync.dma_start(out_dbg[0:1, 0:1], medz[0:1, 0:1])
        nc.sync.dma_start(out_dbg[0:1, 1:2], medn[0:1, 0:1])
        nc.sync.dma_start(out_dbg[0:1, 2:6], ccg[0:1, :])
        nc.sync.dma_start(out_dbg[0:1, 6:7], tz[0:1, 0:1])
        nc.sync.dma_start(out_dbg[0:1, 7:8], tn[0:1, 0:1])

    return nc


def _get_nc():
    if "nc" not in _nc_cache:
        nc = _build()
        _split_waits(nc)
        _nc_cache["nc"] = nc
    return _nc_cache["nc"]


def _sample_median(X32, xsq):
    """Host estimate of the lower-median of the pairwise squared distances."""
    rows = X32[::8]
    cols = X32[::2]
    G = rows @ cols.T
    d2 = xsq[::8, None] + xsq[None, ::2] - 2.0 * G
    flat = d2.ravel()
    return float(np.partition(flat, (flat.size - 1) // 2)[(flat.size - 1) // 2])


def _prepare_inputs(Z, N):
    Zf = np.asarray(Z, dtype=np.float32)
    Nf = np.asarray(N, dtype=np.float32)
    zsq = (Zf.astype(np.float64) ** 2).sum(1).astype(np.float32)
    nsq = (Nf.astype(np.float64) ** 2).sum(1).astype(np.float32)
    Zb = Zf.astype(_BF16)
    Nb = Nf.astype(_BF16)

    def aug(Xb, xsq):
        w = (-0.5 * xsq).astype(np.float32)
        w_hi = w.astype(_BF16)
        w_lo = (w - w_hi.astype(np.float32)).astype(_BF16)
        return np.concatenate(
            [np.ascontiguousarray(Xb.T), w_hi[None, :], w_lo[None, :]], axis=0)

    zt = aug(Zb, zsq)
    nt = aug(Nb, nsq)

    t0z = _sample_median(Zf, zsq)
    t0n = _sample_median(Nf, nsq)
    thr = np.array([t0z - HZ - SH_Z, t0z + HZ - SH_Z,
                    t0n - HN - SH_N, t0n + HN - SH_N], dtype=np.float32)

    in_maps = []
    for c in range(NCORES):
        sl = slice(c * BLK, (c + 1) * BLK)
        in_maps.append({
            "zt": zt,
            "ntr": nt,
            "lhsz": np.ascontiguousarray(Zb.T[:, sl]),
            "lhsn": np.ascontiguousarray(Nb.T[:, sl]),
            "zsqm": (zsq[sl] - SH_Z).astype(np.float32),
            "nsqm": (nsq[sl] - SH_N).astype(np.float32),
            "thr": thr,
        })
    return in_maps


def run_on_device(Z, N, **run_kwargs):
    """Run the bass kernel; returns (BassKernelResults, hsic float)."""
    from concourse.bass_utils import run_bass_kernel_spmd
    nc = _get_nc()
    in_maps = _prepare_inputs(Z, N)
    res = run_bass_kernel_spmd(nc, in_maps, core_ids=list(range(NCORES)),
                               **run_kwargs)
    S = sum(float(r["out_s"][0, 0]) for r in res.results)
    hsic = S / ((NTOT - 1) ** 2 + 1e-8)
    return res, hsic


def kernel(Z, N):
    _, hsic = run_on_device(Z, N)
    return np.asarray(hsic, dtype=np.float32)


if __name__ == "__main__":
    rng = np.random.default_rng(0)
    Z = rng.standard_normal((NTOT, DZ), dtype=np.float32)
    N = rng.standard_normal((NTOT, DN), dtype=np.float32)
    res, hsic = run_on_device(Z, N)
    print("hsic:", hsic)
    print("dbg core0:", res.results[0]["out_dbg"])


# revision 16
# speedup vs baseline: 1.2368x; 1.2368x over previous
"""Distributed HSIC independence loss for Trainium2 (8 NeuronCores).

Pipeline (single NEFF launch, row-sharded across 8 cores):
  1. Per core: P = Zrow @ Zfull.T via TensorE (bf16, f32 accum), with the
     -|z_j|^2/2 term folded in as two extra bf16 contraction rows (hi+lo
     split), so d2 = -2*P + |z_i|^2 comes out of PSUM in one ScalarE
     activation (stored shifted, fp16).
  2. Median of d2: host supplies a sampled estimate t0; the device computes
     exact full counts of d2 <= t0 +/- h, AllReduces the 4 counts (Z and N),
     and linearly interpolates the CDF to get the global lower-median.
  3. K = exp(-d2/(2*sigma^2+1e-8)) via one ScalarE activation per tile
     (runtime per-partition scale/bias), with fused row-sum accumulation.
  4. Row sums are AllGathered; HSIC sum computed via
     sum(Kc*Lc) = sum((v_j - K)(q_j - L)) - n * sum_i alpha_i*beta_i
     with alpha_i = mu_i - mean, so no per-element centering bias passes.
  5. Per-core partials summed on host; divide by (n-1)^2 + 1e-8.
"""

import numpy as np
import ml_dtypes
from contextlib import ExitStack

NCORES = 8
NTOT = 4096
DZ = 512
DN = 128
BLK = NTOT // NCORES      # 512 rows per core
MT = BLK // 128           # 4 M-tiles per core
NB = NTOT // 512          # 8 column tiles of 512
SH_Z = 1024.0             # fp16 storage shift for d2 of Z
SH_N = 256.0
HZ = 8.0                  # count-threshold half-window
HN = 2.0
KTARGET = float((NTOT * NTOT - 1) // 2 + 1)   # 8388608: lower-median rank

_BF16 = ml_dtypes.bfloat16

_nc_cache = {}


def _split_waits(nc, limit=1):
    """This walrus build accepts at most one sync-wait per instruction;
    hoist extra waits onto preceding single-wait drains on the same engine."""
    import concourse.mybir as mybir
    import bass_rust
    ctr = 0
    for f in nc.m.functions:
        for b in f.blocks:
            out, changed = [], False
            for inst in b.instructions:
                si = inst.sync_info
                waits = list(si.on_wait) if si is not None else []
                if len(waits) > limit:
                    changed = True
                    for w in waits[:-limit]:
                        ctr += 1
                        d = mybir.InstDrain(name=f"I-waitsplit-{ctr}", ins=[], outs=[])
                        d.engine = inst.engine
                        d.sync_info = bass_rust.SyncInfo(on_update=[], on_wait=[w])
                        out.append(d)
                    si.on_wait = waits[-limit:]
                out.append(inst)
            if changed:
                b.instructions = out
    return ctr


def _build():
    import concourse.bass as bass
    import concourse.mybir as mybir
    import concourse.tile as tile
    from concourse import bass_isa

    f32 = mybir.dt.float32
    f16 = mybir.dt.float16
    bf16 = mybir.dt.bfloat16
    Alu = mybir.AluOpType
    Act = mybir.ActivationFunctionType
    RG = [list(range(NCORES))]

    nc = bass.Bass("TRN2", num_devices=NCORES)

    zt = nc.dram_tensor("zt", [DZ + 2, NTOT], bf16, kind="ExternalInput")
    ntr = nc.dram_tensor("ntr", [DN + 2, NTOT], bf16, kind="ExternalInput")
    lhsz = nc.dram_tensor("lhsz", [DZ, BLK], bf16, kind="ExternalInput")
    lhsn = nc.dram_tensor("lhsn", [DN, BLK], bf16, kind="ExternalInput")
    zsqm = nc.dram_tensor("zsqm", [BLK], f32, kind="ExternalInput")   # |z_i|^2 - SH_Z
    nsqm = nc.dram_tensor("nsqm", [BLK], f32, kind="ExternalInput")   # |n_i|^2 - SH_N
    thr = nc.dram_tensor("thr", [4], f32, kind="ExternalInput")       # shifted thresholds
    out_s = nc.dram_tensor("out_s", [1, 1], f32, kind="ExternalOutput")
    out_dbg = nc.dram_tensor("out_dbg", [1, 8], f32, kind="ExternalOutput")

    KZT = DZ // 128   # 4 contraction tiles for Z
    KNT = DN // 128   # 1 for N

    with tile.TileContext(nc) as tc, ExitStack() as ctx:
        big = ctx.enter_context(tc.tile_pool(name="big", bufs=1))
        psum = ctx.enter_context(tc.tile_pool(name="psum", bufs=2, space="PSUM"))
        small = ctx.enter_context(tc.tile_pool(name="small", bufs=1))
        dram = ctx.enter_context(tc.tile_pool(name="dram", bufs=1, space="DRAM"))

        # ---------------- input DMAs ----------------
        zt_sb = []
        for k in range(KZT):
            t = big.tile([128, NTOT], bf16, tag=f"zk{k}", name=f"zt_sb{k}")
            nc.sync.dma_start(t[:], zt[k * 128:(k + 1) * 128, :])
            zt_sb.append(t)
        ztw = small.tile([2, NTOT], bf16, tag="ztw", name="ztw")
        nc.sync.dma_start(ztw[:], zt[DZ:DZ + 2, :])

        nt_sb = big.tile([128, NTOT], bf16, tag="nk0", name="nt_sb")
        nc.sync.dma_start(nt_sb[:], ntr[0:128, :])
        ntw = small.tile([2, NTOT], bf16, tag="ntw", name="ntw")
        nc.sync.dma_start(ntw[:], ntr[DN:DN + 2, :])

        lhsz_sb = []
        for k in range(KZT):
            t = small.tile([128, BLK], bf16, tag=f"lz{k}", name=f"lhsz_sb{k}")
            nc.sync.dma_start(t[:], lhsz[k * 128:(k + 1) * 128, :])
            lhsz_sb.append(t)
        lhsn_sb = small.tile([128, BLK], bf16, tag="ln0", name="lhsn_sb")
        nc.sync.dma_start(lhsn_sb[:], lhsn[:, :])

        ones2 = small.tile([2, 128], bf16, tag="ones2", name="ones2")
        nc.vector.memset(ones2[:], 1.0)

        zsqm_sb = small.tile([128, MT], f32, tag="zsqm", name="zsqm_sb")
        nc.sync.dma_start(zsqm_sb[:], zsqm[:].rearrange("(m p) -> p m", p=128))
        nsqm_sb = small.tile([128, MT], f32, tag="nsqm", name="nsqm_sb")
        nc.sync.dma_start(nsqm_sb[:], nsqm[:].rearrange("(m p) -> p m", p=128))

        thrb = small.tile([128, 4], f32, tag="thrb", name="thrb")
        thr_ap = thr[:]
        thr_b = bass.AP(tensor=thr_ap.tensor, offset=thr_ap.offset,
                        ap=[[0, 128], [1, 4]])
        nc.sync.dma_start(thrb[:], thr_b)

        ones1 = small.tile([128, 1], f32, tag="ones1", name="ones1")
        nc.vector.memset(ones1[:], 1.0)

        # ---------------- matmuls + d2s evacuation ----------------
        # d2s laid out as one [128, MT, NTOT] fp16 tile per matrix so later
        # elementwise passes are few, large ops (DVE per-op overhead ~1.5us).
        def mm_phase(d2s, lhs_tiles, rhs_tiles, wtile, sq_sb, kt, mat):
            for m in range(MT):
                ps = [psum.tile([128, 4 * 512], f32, tag="ps",
                                name=f"ps_{mat}{m}_{h}") for h in range(2)]
                for k in range(kt):
                    lw = lhs_tiles[k][:, m * 128:(m + 1) * 128]
                    for nb in range(NB):
                        nc.tensor.matmul(ps[nb // 4][:, (nb % 4) * 512:(nb % 4 + 1) * 512],
                                         lw,
                                         rhs_tiles[k][:, nb * 512:(nb + 1) * 512],
                                         start=(k == 0), stop=False)
                for nb in range(NB):
                    nc.tensor.matmul(ps[nb // 4][:, (nb % 4) * 512:(nb % 4 + 1) * 512],
                                     ones2[:, 0:128],
                                     wtile[:, nb * 512:(nb + 1) * 512],
                                     start=False, stop=True)
                for h in range(2):
                    nc.scalar.activation(d2s[:, m, h * 2048:(h + 1) * 2048],
                                         ps[h][:], Act.Identity,
                                         bias=sq_sb[:, m:m + 1], scale=-2.0)

        def count_pass(engine, d2s_m_ap, thr_ap, scr_ap, acc_ap):
            # count(d2s <= thr) over the even-column subset (x2 on host side)
            engine.tensor_scalar(scr_ap, d2s_m_ap, thr_ap, None,
                                 Alu.is_le, Alu.add, accum_out=acc_ap)

        def cdf_collective(cnt2, mat):
            # cnt2: [128, 2] per-partition counts -> global totals on all parts
            cp = psum.tile([2, 1], f32, tag="ps", name=f"cp_{mat}", bufs=None)
            nc.tensor.matmul(cp[:], cnt2, ones1[:], start=True, stop=True)
            cs = small.tile([2, 1], f32, tag=f"cs_{mat}", name=f"cs_{mat}")
            nc.scalar.activation(cs[:], cp[:], Act.Identity)
            cin = dram.tile([1, 2], f32, tag=f"cin_{mat}", name=f"cin_{mat}")
            cout = dram.tile([1, 2], f32, tag=f"cout_{mat}", name=f"cout_{mat}")
            cin_ap = cin[:]
            nc.sync.dma_start(
                bass.AP(tensor=cin_ap.tensor, offset=cin_ap.offset,
                        ap=[[1, 2], [2, 1]]), cs[:])
            nc.gpsimd.collective_compute("AllReduce", Alu.add, replica_groups=RG,
                                         ins=[cin[:]], outs=[cout[:]])
            cg = small.tile([128, 2], f32, tag=f"cg_{mat}", name=f"cg_{mat}")
            cout_ap = cout[:]
            nc.sync.dma_start(
                cg[:], bass.AP(tensor=cout_ap.tensor, offset=cout_ap.offset,
                               ap=[[0, 128], [1, 2]]))
            return cg

        scr16 = big.tile([128, MT, NTOT], f16, tag="scr", name="scr16")

        # --- N matrix first: its count->AllReduce->exp->AllGather chain
        # overlaps with the Z matmuls ---
        d2sn = big.tile([128, MT, NTOT], f16, tag="dn", name="d2sn")
        mm_phase(d2sn, [lhsn_sb], [nt_sb], ntw, nsqm_sb, KNT, "n")

        CSTRIDE = 4   # count every 4th column; rank target scales by 1/4

        def strided(ap3):
            # [128, MT, NTOT/CSTRIDE] strided-column view of the whole tile
            sl = ap3[:].rearrange("p m (j s) -> p m j s", s=CSTRIDE)
            return sl[:, :, :, 0]

        cn = small.tile([128, 2], f32, tag="cn", name="cn")
        for t in range(2):
            count_pass(nc.vector, strided(d2sn), thrb[:, 2 + t:3 + t],
                       scr16[:, :, t * 1024:(t + 1) * 1024], cn[:, t:t + 1])
        cgn = cdf_collective(cn[:], "n")

        # --- Z matrix ---
        d2sz = big.tile([128, MT, NTOT], f16, tag="dz", name="d2sz")
        mm_phase(d2sz, lhsz_sb, zt_sb, ztw, zsqm_sb, KZT, "z")

        cz = small.tile([128, 2], f32, tag="cz", name="cz")
        for t in range(2):
            count_pass(nc.vector, strided(d2sz), thrb[:, t:t + 1],
                       scr16[:, :, 2048 + t * 1024:2048 + (t + 1) * 1024],
                       cz[:, t:t + 1])
        cgz = cdf_collective(cz[:], "z")

        # ---------------- median interpolation + exp coefficients ----------------
        # counts cover the even-column half of the matrix -> rank target k/2
        def interp(c0, c1, t0ap, h, shift, mat):
            num = small.tile([128, 1], f32, tag=f"num{mat}", name=f"num{mat}")
            nc.vector.tensor_scalar(num[:], c0, KTARGET / 4.0, -1.0, Alu.subtract,
                                    Alu.mult)                  # (C0-k)*-1 = k-C0
            den = small.tile([128, 1], f32, tag=f"den{mat}", name=f"den{mat}")
            nc.vector.tensor_sub(den[:], c1, c0)
            rec = small.tile([128, 1], f32, tag=f"rec{mat}", name=f"rec{mat}")
            nc.vector.reciprocal(rec[:], den[:])
            r = small.tile([128, 1], f32, tag=f"r{mat}", name=f"r{mat}")
            nc.vector.tensor_mul(r[:], num[:], rec[:])
            rc = small.tile([128, 1], f32, tag=f"rc{mat}", name=f"rc{mat}")
            nc.vector.tensor_scalar(rc[:], r[:], 0.0, 1.0, Alu.max, Alu.min)
            meds = small.tile([128, 1], f32, tag=f"meds{mat}", name=f"meds{mat}")
            nc.vector.tensor_scalar(meds[:], rc[:], 2.0 * h, t0ap, Alu.mult, Alu.add)
            tmp = small.tile([128, 1], f32, tag=f"tmp{mat}", name=f"tmp{mat}")
            nc.vector.tensor_scalar(tmp[:], meds[:], shift + 3e-8, None, Alu.add)
            s = small.tile([128, 1], f32, tag=f"s{mat}", name=f"s{mat}")
            nc.vector.reciprocal(s[:], tmp[:])
            sc = small.tile([128, 1], f32, tag=f"sc{mat}", name=f"sc{mat}")
            nc.vector.tensor_scalar(sc[:], s[:], -1.0, None, Alu.mult)
            bs = small.tile([128, 1], f32, tag=f"bs{mat}", name=f"bs{mat}")
            nc.vector.tensor_scalar(bs[:], s[:], -shift, None, Alu.mult)
            return meds, sc, bs

        medn, scn, bsn = interp(cgn[:, 0:1], cgn[:, 1:2], thrb[:, 2:3], HN, SH_N, "n")
        medz, scz, bsz = interp(cgz[:, 0:1], cgz[:, 1:2], thrb[:, 0:1], HZ, SH_Z, "z")

        # ---------------- exp (in place, d2s becomes K) + fused row sums;
        # per-matrix AllGather of row sums, broadcast to all partitions -------
        def exp_gather(d2s, sc, bs, mat):
            r = small.tile([128, MT], f32, tag=f"r{mat}x", name=f"r{mat}x")
            for m in range(MT):
                nc.scalar.activation(d2s[:, m, :], d2s[:, m, :], Act.Exp,
                                     bias=bs[:], scale=sc[:],
                                     accum_out=r[:, m:m + 1])
            agi = dram.tile([1, BLK], f32, tag=f"agi_{mat}", name=f"agi_{mat}")
            ago = dram.tile([NCORES, BLK], f32, tag=f"ago_{mat}", name=f"ago_{mat}")
            agi_ap = agi[:]
            nc.sync.dma_start(
                bass.AP(tensor=agi_ap.tensor, offset=agi_ap.offset,
                        ap=[[1, 128], [128, MT]]), r[:])
            nc.gpsimd.collective_compute("AllGather", Alu.bypass,
                                         replica_groups=RG,
                                         ins=[agi[:]], outs=[ago[:]])
            v = big.tile([128, NCORES, BLK], f32, tag=f"v{mat}", name=f"v{mat}")
            ago_ap = ago[:]
            nc.sync.dma_start(
                v[:], bass.AP(tensor=ago_ap.tensor, offset=ago_ap.offset,
                              ap=[[0, 128], [BLK, NCORES], [1, BLK]]))
            return r, v

        rn, vn = exp_gather(d2sn, scn, bsn, "n")
        rz, vz = exp_gather(d2sz, scz, bsz, "z")

        # ---------------- means / alpha / correction ----------------
        inv_n = 1.0 / NTOT
        inv_n2 = 1.0 / (NTOT * NTOT)
        tz = small.tile([128, 1], f32, tag="tz", name="tz")
        nc.vector.tensor_reduce(tz[:], vz[:], mybir.AxisListType.XY, Alu.add)
        tn = small.tile([128, 1], f32, tag="tn", name="tn")
        nc.vector.tensor_reduce(tn[:], vn[:], mybir.AxisListType.XY, Alu.add)
        mbz = small.tile([128, 1], f32, tag="mbz", name="mbz")
        nc.vector.tensor_scalar(mbz[:], tz[:], inv_n2, None, Alu.mult)
        mbn = small.tile([128, 1], f32, tag="mbn", name="mbn")
        nc.vector.tensor_scalar(mbn[:], tn[:], inv_n2, None, Alu.mult)
        az = small.tile([128, MT], f32, tag="az", name="az")
        nc.vector.tensor_scalar(az[:], rz[:], inv_n, mbz[:], Alu.mult, Alu.subtract)
        an = small.tile([128, MT], f32, tag="an", name="an")
        nc.vector.tensor_scalar(an[:], rn[:], inv_n, mbn[:], Alu.mult, Alu.subtract)
        ca = small.tile([128, MT], f32, tag="ca", name="ca")
        nc.vector.tensor_mul(ca[:], az[:], an[:])

        # ---------------- final centered product ----------------
        # A = v_j - K (in place over K), B = q_j - L (gpsimd, in place over L),
        # then one fused product+accumulate pass:
        #   S_local = sum(A*B) - n * sum_i alpha_i beta_i
        def v_bcast(v):
            vf = v[:].rearrange("p c b -> p (c b)")
            return bass.AP(tensor=vf.tensor, offset=vf.offset,
                           ap=[vf.ap[0], [0, MT], vf.ap[1]])

        nc.vector.scalar_tensor_tensor(d2sz[:], v_bcast(vz), inv_n, d2sz[:],
                                       Alu.mult, Alu.subtract)
        nc.vector.scalar_tensor_tensor(d2sn[:], v_bcast(vn), inv_n, d2sn[:],
                                       Alu.mult, Alu.subtract)
        pacc = small.tile([128, 1], f32, tag="pacc", name="pacc")
        nc.vector.scalar_tensor_tensor(
            scr16[:], d2sz[:], 1.0, d2sn[:], Alu.mult, Alu.mult,
            accum_out=pacc[:, 0:1])

        cb = small.tile([128, 1], f32, tag="cb", name="cb")
        nc.vector.tensor_reduce(cb[:], ca[:], mybir.AxisListType.X, Alu.add)
        spp = small.tile([128, 1], f32, tag="spp", name="spp")
        nc.vector.tensor_scalar(spp[:], cb[:], -float(NTOT), pacc[:],
                                Alu.mult, Alu.add)
        stp = psum.tile([1, 1], f32, tag="ps", name="stp")
        nc.tensor.matmul(stp[:], spp[:], ones1[:], start=True, stop=True)
        stot = small.tile([1, 1], f32, tag="stot", name="stot")
        nc.scalar.activation(stot[:], stp[:], Act.Identity)
        nc.sync.dma_start(out_s[:], stot[0:1, 0:1])

        # debug outputs
        nc.sync.dma_start(out_dbg[0:1, 0:1], medz[0:1, 0:1])
        nc.sync.dma_start(out_dbg[0:1, 1:2], medn[0:1, 0:1])
        nc.sync.dma_start(out_dbg[0:1, 2:4], cgz[0:1, :])
        nc.sync.dma_start(out_dbg[0:1, 4:6], cgn[0:1, :])
        nc.sync.dma_start(out_dbg[0:1, 6:7], tz[0:1, 0:1])
        nc.sync.dma_start(out_dbg[0:1, 7:8], tn[0:1, 0:1])

    return nc


def _get_nc():
    if "nc" not in _nc_cache:
        nc = _build()
        _split_waits(nc)
        _nc_cache["nc"] = nc
    return _nc_cache["nc"]


def _sample_median(X32, xsq):
    """Host estimate of the lower-median of the pairwise squared distances."""
    rows = X32[::8]
    cols = X32[::2]
    G = rows @ cols.T
    d2 = xsq[::8, None] + xsq[None, ::2] - 2.0 * G
    flat = d2.ravel()
    return float(np.partition(flat, (flat.size - 1) // 2)[(flat.size - 1) // 2])


def _prepare_inputs(Z, N):
    Zf = np.asarray(Z, dtype=np.float32)
    Nf = np.asarray(N, dtype=np.float32)
    zsq = (Zf.astype(np.float64) ** 2).sum(1).astype(np.float32)
    nsq = (Nf.astype(np.float64) ** 2).sum(1).astype(np.float32)
    Zb = Zf.astype(_BF16)
    Nb = Nf.astype(_BF16)

    def aug(Xb, xsq):
        w = (-0.5 * xsq).astype(np.float32)
        w_hi = w.astype(_BF16)
        w_lo = (w - w_hi.astype(np.float32)).astype(_BF16)
        return np.concatenate(
            [np.ascontiguousarray(Xb.T), w_hi[None, :], w_lo[None, :]], axis=0)

    zt = aug(Zb, zsq)
    nt = aug(Nb, nsq)

    t0z = _sample_median(Zf, zsq)
    t0n = _sample_median(Nf, nsq)
    thr = np.array([t0z - HZ - SH_Z, t0z + HZ - SH_Z,
                    t0n - HN - SH_N, t0n + HN - SH_N], dtype=np.float32)
    # keep thresholds off the fp16 grid so is_le sees no exact ties
    on_grid = thr == thr.astype(np.float16).astype(np.float32)
    thr[on_grid] += np.float32(1.001953125e-3)

    in_maps = []
    for c in range(NCORES):
        sl = slice(c * BLK, (c + 1) * BLK)
        in_maps.append({
            "zt": zt,
            "ntr": nt,
            "lhsz": np.ascontiguousarray(Zb.T[:, sl]),
            "lhsn": np.ascontiguousarray(Nb.T[:, sl]),
            "zsqm": (zsq[sl] - SH_Z).astype(np.float32),
            "nsqm": (nsq[sl] - SH_N).astype(np.float32),
            "thr": thr,
        })
    return in_maps


def run_on_device(Z, N, **run_kwargs):
    """Run the bass kernel; returns (BassKernelResults, hsic float)."""
    from concourse.bass_utils import run_bass_kernel_spmd
    nc = _get_nc()
    in_maps = _prepare_inputs(Z, N)
    res = run_bass_kernel_spmd(nc, in_maps, core_ids=list(range(NCORES)),
                               **run_kwargs)
    S = sum(float(r["out_s"][0, 0]) for r in res.results)
    hsic = S / ((NTOT - 1) ** 2 + 1e-8)
    return res, hsic


def kernel(Z, N):
    _, hsic = run_on_device(Z, N)
    return np.asarray(hsic, dtype=np.float32)


if __name__ == "__main__":
    rng = np.random.default_rng(0)
    Z = rng.standard_normal((NTOT, DZ), dtype=np.float32)
    N = rng.standard_normal((NTOT, DN), dtype=np.float32)
    res, hsic = run_on_device(Z, N)
    print("hsic:", hsic)
    print("dbg core0:", res.results[0]["out_dbg"])


# revision 19
# speedup vs baseline: 1.2420x; 1.0042x over previous
"""Distributed HSIC independence loss for Trainium2 (8 NeuronCores).

Pipeline (single NEFF launch, row-sharded across 8 cores):
  1. Per core: P = Zrow @ Zfull.T via TensorE (bf16, f32 accum), with the
     -|z_j|^2/2 term folded in as two extra bf16 contraction rows (hi+lo
     split), so d2 = -2*P + |z_i|^2 comes out of PSUM in one ScalarE
     activation (stored shifted, fp16).
  2. Median of d2: host supplies a sampled estimate t0; the device computes
     exact full counts of d2 <= t0 +/- h, AllReduces the 4 counts (Z and N),
     and linearly interpolates the CDF to get the global lower-median.
  3. K = exp(-d2/(2*sigma^2+1e-8)) via one ScalarE activation per tile
     (runtime per-partition scale/bias), with fused row-sum accumulation.
  4. Row sums are AllGathered; HSIC sum computed via
     sum(Kc*Lc) = sum((v_j - K)(q_j - L)) - n * sum_i alpha_i*beta_i
     with alpha_i = mu_i - mean, so no per-element centering bias passes.
  5. Per-core partials summed on host; divide by (n-1)^2 + 1e-8.
"""

import numpy as np
import ml_dtypes
from contextlib import ExitStack

NCORES = 8
NTOT = 4096
DZ = 512
DN = 128
BLK = NTOT // NCORES      # 512 rows per core
MT = BLK // 128           # 4 M-tiles per core
NB = NTOT // 512          # 8 column tiles of 512
SH_Z = 1024.0             # fp16 storage shift for d2 of Z
SH_N = 256.0
HZ = 10.0                 # count-threshold half-window
HN = 2.5
KTARGET = float((NTOT * NTOT - 1) // 2 + 1)   # 8388608: lower-median rank

_BF16 = ml_dtypes.bfloat16

_nc_cache = {}


def _split_waits(nc, limit=1):
    """This walrus build accepts at most one sync-wait per instruction;
    hoist extra waits onto preceding single-wait drains on the same engine."""
    import concourse.mybir as mybir
    import bass_rust
    ctr = 0
    for f in nc.m.functions:
        for b in f.blocks:
            out, changed = [], False
            for inst in b.instructions:
                si = inst.sync_info
                waits = list(si.on_wait) if si is not None else []
                if len(waits) > limit:
                    changed = True
                    for w in waits[:-limit]:
                        ctr += 1
                        d = mybir.InstDrain(name=f"I-waitsplit-{ctr}", ins=[], outs=[])
                        d.engine = inst.engine
                        d.sync_info = bass_rust.SyncInfo(on_update=[], on_wait=[w])
                        out.append(d)
                    si.on_wait = waits[-limit:]
                out.append(inst)
            if changed:
                b.instructions = out
    return ctr


def _build():
    import concourse.bass as bass
    import concourse.mybir as mybir
    import concourse.tile as tile
    from concourse import bass_isa

    f32 = mybir.dt.float32
    f16 = mybir.dt.float16
    bf16 = mybir.dt.bfloat16
    Alu = mybir.AluOpType
    Act = mybir.ActivationFunctionType
    RG = [list(range(NCORES))]

    nc = bass.Bass("TRN2", num_devices=NCORES)

    zt = nc.dram_tensor("zt", [DZ + 2, NTOT], bf16, kind="ExternalInput")
    ntr = nc.dram_tensor("ntr", [DN + 2, NTOT], bf16, kind="ExternalInput")
    lhsz = nc.dram_tensor("lhsz", [DZ, BLK], bf16, kind="ExternalInput")
    lhsn = nc.dram_tensor("lhsn", [DN, BLK], bf16, kind="ExternalInput")
    zsqm = nc.dram_tensor("zsqm", [BLK], f32, kind="ExternalInput")   # |z_i|^2 - SH_Z
    nsqm = nc.dram_tensor("nsqm", [BLK], f32, kind="ExternalInput")   # |n_i|^2 - SH_N
    thr = nc.dram_tensor("thr", [4], f32, kind="ExternalInput")       # shifted thresholds
    out_s = nc.dram_tensor("out_s", [1, 1], f32, kind="ExternalOutput")
    out_dbg = nc.dram_tensor("out_dbg", [1, 8], f32, kind="ExternalOutput")

    KZT = DZ // 128   # 4 contraction tiles for Z
    KNT = DN // 128   # 1 for N

    with tile.TileContext(nc) as tc, ExitStack() as ctx:
        big = ctx.enter_context(tc.tile_pool(name="big", bufs=1))
        psum = ctx.enter_context(tc.tile_pool(name="psum", bufs=2, space="PSUM"))
        small = ctx.enter_context(tc.tile_pool(name="small", bufs=1))
        dram = ctx.enter_context(tc.tile_pool(name="dram", bufs=1, space="DRAM"))

        # ---------------- input DMAs ----------------
        zt_sb = []
        for k in range(KZT):
            t = big.tile([128, NTOT], bf16, tag=f"zk{k}", name=f"zt_sb{k}")
            nc.sync.dma_start(t[:], zt[k * 128:(k + 1) * 128, :])
            zt_sb.append(t)
        ztw = small.tile([2, NTOT], bf16, tag="ztw", name="ztw")
        nc.sync.dma_start(ztw[:], zt[DZ:DZ + 2, :])

        nt_sb = big.tile([128, NTOT], bf16, tag="nk0", name="nt_sb")
        nc.sync.dma_start(nt_sb[:], ntr[0:128, :])
        ntw = small.tile([2, NTOT], bf16, tag="ntw", name="ntw")
        nc.sync.dma_start(ntw[:], ntr[DN:DN + 2, :])

        lhsz_sb = []
        for k in range(KZT):
            t = small.tile([128, BLK], bf16, tag=f"lz{k}", name=f"lhsz_sb{k}")
            nc.sync.dma_start(t[:], lhsz[k * 128:(k + 1) * 128, :])
            lhsz_sb.append(t)
        lhsn_sb = small.tile([128, BLK], bf16, tag="ln0", name="lhsn_sb")
        nc.sync.dma_start(lhsn_sb[:], lhsn[:, :])

        ones2 = small.tile([2, 128], bf16, tag="ones2", name="ones2")
        nc.vector.memset(ones2[:], 1.0)

        zsqm_sb = small.tile([128, MT], f32, tag="zsqm", name="zsqm_sb")
        nc.sync.dma_start(zsqm_sb[:], zsqm[:].rearrange("(m p) -> p m", p=128))
        nsqm_sb = small.tile([128, MT], f32, tag="nsqm", name="nsqm_sb")
        nc.sync.dma_start(nsqm_sb[:], nsqm[:].rearrange("(m p) -> p m", p=128))

        thrb = small.tile([128, 4], f32, tag="thrb", name="thrb")
        thr_ap = thr[:]
        thr_b = bass.AP(tensor=thr_ap.tensor, offset=thr_ap.offset,
                        ap=[[0, 128], [1, 4]])
        nc.sync.dma_start(thrb[:], thr_b)

        ones1 = small.tile([128, 1], f32, tag="ones1", name="ones1")
        nc.vector.memset(ones1[:], 1.0)

        # ---------------- matmuls + d2s evacuation ----------------
        # d2s laid out as one [128, MT, NTOT] fp16 tile per matrix so later
        # elementwise passes are few, large ops (DVE per-op overhead ~1.5us).
        def mm_phase(d2s, lhs_tiles, rhs_tiles, wtile, sq_sb, kt, mat):
            for m in range(MT):
                ps = [psum.tile([128, 4 * 512], f32, tag="ps",
                                name=f"ps_{mat}{m}_{h}") for h in range(2)]
                for k in range(kt):
                    lw = lhs_tiles[k][:, m * 128:(m + 1) * 128]
                    for nb in range(NB):
                        nc.tensor.matmul(ps[nb // 4][:, (nb % 4) * 512:(nb % 4 + 1) * 512],
                                         lw,
                                         rhs_tiles[k][:, nb * 512:(nb + 1) * 512],
                                         start=(k == 0), stop=False)
                for nb in range(NB):
                    nc.tensor.matmul(ps[nb // 4][:, (nb % 4) * 512:(nb % 4 + 1) * 512],
                                     ones2[:, 0:128],
                                     wtile[:, nb * 512:(nb + 1) * 512],
                                     start=False, stop=True)
                for h in range(2):
                    nc.scalar.activation(d2s[:, m, h * 2048:(h + 1) * 2048],
                                         ps[h][:], Act.Identity,
                                         bias=sq_sb[:, m:m + 1], scale=-2.0)

        def count_pass(engine, d2s_m_ap, thr_ap, scr_ap, acc_ap):
            # count(d2s <= thr) over the even-column subset (x2 on host side)
            engine.tensor_scalar(scr_ap, d2s_m_ap, thr_ap, None,
                                 Alu.is_le, Alu.add, accum_out=acc_ap)

        def cdf_collective(cnt2, mat):
            # cnt2: [128, 2] per-partition counts -> global totals on all parts
            cp = psum.tile([2, 1], f32, tag="ps", name=f"cp_{mat}", bufs=None)
            nc.tensor.matmul(cp[:], cnt2, ones1[:], start=True, stop=True)
            cs = small.tile([2, 1], f32, tag=f"cs_{mat}", name=f"cs_{mat}")
            nc.scalar.activation(cs[:], cp[:], Act.Identity)
            cin = dram.tile([1, 2], f32, tag=f"cin_{mat}", name=f"cin_{mat}")
            cout = dram.tile([1, 2], f32, tag=f"cout_{mat}", name=f"cout_{mat}")
            cin_ap = cin[:]
            nc.sync.dma_start(
                bass.AP(tensor=cin_ap.tensor, offset=cin_ap.offset,
                        ap=[[1, 2], [2, 1]]), cs[:])
            nc.gpsimd.collective_compute("AllReduce", Alu.add, replica_groups=RG,
                                         ins=[cin[:]], outs=[cout[:]])
            cg = small.tile([128, 2], f32, tag=f"cg_{mat}", name=f"cg_{mat}")
            cout_ap = cout[:]
            nc.sync.dma_start(
                cg[:], bass.AP(tensor=cout_ap.tensor, offset=cout_ap.offset,
                               ap=[[0, 128], [1, 2]]))
            return cg

        scr16 = big.tile([128, MT, NTOT], f16, tag="scr", name="scr16")

        # --- N matrix first: its count->AllReduce->exp->AllGather chain
        # overlaps with the Z matmuls ---
        d2sn = big.tile([128, MT, NTOT], f16, tag="dn", name="d2sn")
        mm_phase(d2sn, [lhsn_sb], [nt_sb], ntw, nsqm_sb, KNT, "n")

        CSTRIDE = 4   # count every 4th column; rank target scales by 1/4
        nthrb = small.tile([128, 4], f32, tag="nthrb", name="nthrb")
        nc.vector.tensor_scalar(nthrb[:], thrb[:], -1.0, None, Alu.mult)

        def strided(ap3, m):
            # every 4th column of m-slice, phase m%4 so that across the four
            # m-tiles every column is sampled equally (unbiased CDF sample)
            sl = ap3[:, m, :].rearrange("p (j s) -> p j s", s=CSTRIDE)
            return sl[:, :, m % CSTRIDE]

        def counts(d2s, thr_lo_col, mat):
            # thr_lo via DVE is_le; thr_hi via ScalarE Sign (count = 2048 - sg/2)
            clo = small.tile([128, MT], f32, tag=f"clo_{mat}", name=f"clo_{mat}")
            chi = small.tile([128, MT], f32, tag=f"chi_{mat}", name=f"chi_{mat}")
            for m in range(MT):
                count_pass(nc.vector, strided(d2s, m), thrb[:, thr_lo_col:thr_lo_col + 1],
                           scr16[:, m, 0:1024], clo[:, m:m + 1])
                count_pass(nc.vector, strided(d2s, m),
                           thrb[:, thr_lo_col + 1:thr_lo_col + 2],
                           scr16[:, m, 1024:2048], chi[:, m:m + 1])
            c2 = small.tile([128, 2], f32, tag=f"c2_{mat}", name=f"c2_{mat}")
            nc.vector.tensor_reduce(c2[:, 0:1], clo[:], mybir.AxisListType.X, Alu.add)
            nc.vector.tensor_reduce(c2[:, 1:2], chi[:], mybir.AxisListType.X, Alu.add)
            return cdf_collective(c2[:], mat)

        cgn = counts(d2sn, 2, "n")

        # --- Z matrix ---
        d2sz = big.tile([128, MT, NTOT], f16, tag="dz", name="d2sz")
        mm_phase(d2sz, lhsz_sb, zt_sb, ztw, zsqm_sb, KZT, "z")

        cgz = counts(d2sz, 0, "z")

        # ---------------- median interpolation + exp coefficients ----------------
        # counts cover the even-column half of the matrix -> rank target k/2
        def interp(c0, c1, t0ap, h, shift, mat):
            num = small.tile([128, 1], f32, tag=f"num{mat}", name=f"num{mat}")
            nc.vector.tensor_scalar(num[:], c0, KTARGET / 4.0, -1.0, Alu.subtract,
                                    Alu.mult)                  # (C0-k)*-1 = k-C0
            den = small.tile([128, 1], f32, tag=f"den{mat}", name=f"den{mat}")
            nc.vector.tensor_sub(den[:], c1, c0)
            rec = small.tile([128, 1], f32, tag=f"rec{mat}", name=f"rec{mat}")
            nc.vector.reciprocal(rec[:], den[:])
            r = small.tile([128, 1], f32, tag=f"r{mat}", name=f"r{mat}")
            nc.vector.tensor_mul(r[:], num[:], rec[:])
            rc = small.tile([128, 1], f32, tag=f"rc{mat}", name=f"rc{mat}")
            nc.vector.tensor_scalar(rc[:], r[:], 0.0, 1.0, Alu.max, Alu.min)
            meds = small.tile([128, 1], f32, tag=f"meds{mat}", name=f"meds{mat}")
            nc.vector.tensor_scalar(meds[:], rc[:], 2.0 * h, t0ap, Alu.mult, Alu.add)
            tmp = small.tile([128, 1], f32, tag=f"tmp{mat}", name=f"tmp{mat}")
            nc.vector.tensor_scalar(tmp[:], meds[:], shift + 3e-8, None, Alu.add)
            s = small.tile([128, 1], f32, tag=f"s{mat}", name=f"s{mat}")
            nc.vector.reciprocal(s[:], tmp[:])
            sc = small.tile([128, 1], f32, tag=f"sc{mat}", name=f"sc{mat}")
            nc.vector.tensor_scalar(sc[:], s[:], -1.0, None, Alu.mult)
            bs = small.tile([128, 1], f32, tag=f"bs{mat}", name=f"bs{mat}")
            nc.vector.tensor_scalar(bs[:], s[:], -shift, None, Alu.mult)
            return meds, sc, bs

        medn, scn, bsn = interp(cgn[:, 0:1], cgn[:, 1:2], thrb[:, 2:3], HN, SH_N, "n")
        medz, scz, bsz = interp(cgz[:, 0:1], cgz[:, 1:2], thrb[:, 0:1], HZ, SH_Z, "z")

        # ---------------- exp (in place, d2s becomes K) + fused row sums;
        # per-matrix AllGather of row sums, broadcast to all partitions -------
        def exp_gather(d2s, sc, bs, mat):
            r = small.tile([128, MT], f32, tag=f"r{mat}x", name=f"r{mat}x")
            for m in range(MT):
                nc.scalar.activation(d2s[:, m, :], d2s[:, m, :], Act.Exp,
                                     bias=bs[:], scale=sc[:],
                                     accum_out=r[:, m:m + 1])
            agi = dram.tile([1, BLK], f32, tag=f"agi_{mat}", name=f"agi_{mat}")
            ago = dram.tile([NCORES, BLK], f32, tag=f"ago_{mat}", name=f"ago_{mat}")
            agi_ap = agi[:]
            nc.sync.dma_start(
                bass.AP(tensor=agi_ap.tensor, offset=agi_ap.offset,
                        ap=[[1, 128], [128, MT]]), r[:])
            nc.gpsimd.collective_compute("AllGather", Alu.bypass,
                                         replica_groups=RG,
                                         ins=[agi[:]], outs=[ago[:]])
            v = big.tile([128, NCORES, BLK], f32, tag=f"v{mat}", name=f"v{mat}")
            ago_ap = ago[:]
            nc.sync.dma_start(
                v[:], bass.AP(tensor=ago_ap.tensor, offset=ago_ap.offset,
                              ap=[[0, 128], [BLK, NCORES], [1, BLK]]))
            return r, v

        rn, vn = exp_gather(d2sn, scn, bsn, "n")
        rz, vz = exp_gather(d2sz, scz, bsz, "z")

        # ---------------- means / alpha / correction ----------------
        inv_n = 1.0 / NTOT
        inv_n2 = 1.0 / (NTOT * NTOT)
        tz = small.tile([128, 1], f32, tag="tz", name="tz")
        nc.vector.tensor_reduce(tz[:], vz[:], mybir.AxisListType.XY, Alu.add)
        tn = small.tile([128, 1], f32, tag="tn", name="tn")
        nc.vector.tensor_reduce(tn[:], vn[:], mybir.AxisListType.XY, Alu.add)
        mbz = small.tile([128, 1], f32, tag="mbz", name="mbz")
        nc.vector.tensor_scalar(mbz[:], tz[:], inv_n2, None, Alu.mult)
        mbn = small.tile([128, 1], f32, tag="mbn", name="mbn")
        nc.vector.tensor_scalar(mbn[:], tn[:], inv_n2, None, Alu.mult)
        az = small.tile([128, MT], f32, tag="az", name="az")
        nc.vector.tensor_scalar(az[:], rz[:], inv_n, mbz[:], Alu.mult, Alu.subtract)
        an = small.tile([128, MT], f32, tag="an", name="an")
        nc.vector.tensor_scalar(an[:], rn[:], inv_n, mbn[:], Alu.mult, Alu.subtract)
        ca = small.tile([128, MT], f32, tag="ca", name="ca")
        nc.vector.tensor_mul(ca[:], az[:], an[:])

        # ---------------- final centered product ----------------
        # A = v_j - K (in place over K), B = q_j - L (gpsimd, in place over L),
        # then one fused product+accumulate pass:
        #   S_local = sum(A*B) - n * sum_i alpha_i beta_i
        def v_bcast(v):
            vf = v[:].rearrange("p c b -> p (c b)")
            return bass.AP(tensor=vf.tensor, offset=vf.offset,
                           ap=[vf.ap[0], [0, MT], vf.ap[1]])

        nc.vector.scalar_tensor_tensor(d2sz[:], v_bcast(vz), inv_n, d2sz[:],
                                       Alu.mult, Alu.subtract)
        nc.vector.scalar_tensor_tensor(d2sn[:], v_bcast(vn), inv_n, d2sn[:],
                                       Alu.mult, Alu.subtract)
        pacc = small.tile([128, 1], f32, tag="pacc", name="pacc")
        nc.vector.scalar_tensor_tensor(
            scr16[:], d2sz[:], 1.0, d2sn[:], Alu.mult, Alu.mult,
            accum_out=pacc[:, 0:1])

        cb = small.tile([128, 1], f32, tag="cb", name="cb")
        nc.vector.tensor_reduce(cb[:], ca[:], mybir.AxisListType.X, Alu.add)
        spp = small.tile([128, 1], f32, tag="spp", name="spp")
        nc.vector.tensor_scalar(spp[:], cb[:], -float(NTOT), pacc[:],
                                Alu.mult, Alu.add)
        stp = psum.tile([1, 1], f32, tag="ps", name="stp")
        nc.tensor.matmul(stp[:], spp[:], ones1[:], start=True, stop=True)
        stot = small.tile([1, 1], f32, tag="stot", name="stot")
        nc.scalar.activation(stot[:], stp[:], Act.Identity)
        nc.sync.dma_start(out_s[:], stot[0:1, 0:1])

        # debug outputs
        nc.sync.dma_start(out_dbg[0:1, 0:1], medz[0:1, 0:1])
        nc.sync.dma_start(out_dbg[0:1, 1:2], medn[0:1, 0:1])
        nc.sync.dma_start(out_dbg[0:1, 2:4], cgz[0:1, :])
        nc.sync.dma_start(out_dbg[0:1, 4:6], cgn[0:1, :])
        nc.sync.dma_start(out_dbg[0:1, 6:7], tz[0:1, 0:1])
        nc.sync.dma_start(out_dbg[0:1, 7:8], tn[0:1, 0:1])

    return nc


def _get_nc():
    if "nc" not in _nc_cache:
        nc = _build()
        _split_waits(nc)
        _nc_cache["nc"] = nc
    return _nc_cache["nc"]


def _sample_median(X32, xsq):
    """Host estimate of the lower-median of the pairwise squared distances."""
    rows = X32[::8]
    cols = X32[::2]
    G = rows @ cols.T
    d2 = xsq[::8, None] + xsq[None, ::2] - 2.0 * G
    flat = d2.ravel()
    return float(np.partition(flat, (flat.size - 1) // 2)[(flat.size - 1) // 2])


def _prepare_inputs(Z, N):
    Zf = np.asarray(Z, dtype=np.float32)
    Nf = np.asarray(N, dtype=np.float32)
    zsq = (Zf.astype(np.float64) ** 2).sum(1).astype(np.float32)
    nsq = (Nf.astype(np.float64) ** 2).sum(1).astype(np.float32)
    Zb = Zf.astype(_BF16)
    Nb = Nf.astype(_BF16)

    def aug(Xb, xsq):
        w = (-0.5 * xsq).astype(np.float32)
        w_hi = w.astype(_BF16)
        w_lo = (w - w_hi.astype(np.float32)).astype(_BF16)
        return np.concatenate(
            [np.ascontiguousarray(Xb.T), w_hi[None, :], w_lo[None, :]], axis=0)

    zt = aug(Zb, zsq)
    nt = aug(Nb, nsq)

    t0z = _sample_median(Zf, zsq)
    t0n = _sample_median(Nf, nsq)
    thr = np.array([t0z - HZ - SH_Z, t0z + HZ - SH_Z,
                    t0n - HN - SH_N, t0n + HN - SH_N], dtype=np.float32)
    # keep thresholds off the fp16 grid so is_le sees no exact ties
    on_grid = thr == thr.astype(np.float16).astype(np.float32)
    thr[on_grid] += np.float32(1.001953125e-3)

    in_maps = []
    for c in range(NCORES):
        sl = slice(c * BLK, (c + 1) * BLK)
        in_maps.append({
            "zt": zt,
            "ntr": nt,
            "lhsz": np.ascontiguousarray(Zb.T[:, sl]),
            "lhsn": np.ascontiguousarray(Nb.T[:, sl]),
            "zsqm": (zsq[sl] - SH_Z).astype(np.float32),
            "nsqm": (nsq[sl] - SH_N).astype(np.float32),
            "thr": thr,
        })
    return in_maps


def run_on_device(Z, N, **run_kwargs):
    """Run the bass kernel; returns (BassKernelResults, hsic float)."""
    from concourse.bass_utils import run_bass_kernel_spmd
    nc = _get_nc()
    in_maps = _prepare_inputs(Z, N)
    res = run_bass_kernel_spmd(nc, in_maps, core_ids=list(range(NCORES)),
                               **run_kwargs)
    S = sum(float(r["out_s"][0, 0]) for r in res.results)
    hsic = S / ((NTOT - 1) ** 2 + 1e-8)
    return res, hsic


def kernel(Z, N):
    _, hsic = run_on_device(Z, N)
    return np.asarray(hsic, dtype=np.float32)


if __name__ == "__main__":
    rng = np.random.default_rng(0)
    Z = rng.standard_normal((NTOT, DZ), dtype=np.float32)
    N = rng.standard_normal((NTOT, DN), dtype=np.float32)
    res, hsic = run_on_device(Z, N)
    print("hsic:", hsic)
    print("dbg core0:", res.results[0]["out_dbg"])


# revision 21
# speedup vs baseline: 1.2492x; 1.0057x over previous
"""Distributed HSIC independence loss for Trainium2 (8 NeuronCores).

Pipeline (single NEFF launch, row-sharded across 8 cores):
  1. Per core: P = Zrow @ Zfull.T via TensorE (bf16, f32 accum), with the
     -|z_j|^2/2 term folded in as two extra bf16 contraction rows (hi+lo
     split), so d2 = -2*P + |z_i|^2 comes out of PSUM in one ScalarE
     activation (stored shifted, fp16).
  2. Median of d2: host supplies a sampled estimate t0; the device computes
     exact full counts of d2 <= t0 +/- h, AllReduces the 4 counts (Z and N),
     and linearly interpolates the CDF to get the global lower-median.
  3. K = exp(-d2/(2*sigma^2+1e-8)) via one ScalarE activation per tile
     (runtime per-partition scale/bias), with fused row-sum accumulation.
  4. Row sums are AllGathered; HSIC sum computed via
     sum(Kc*Lc) = sum((v_j - K)(q_j - L)) - n * sum_i alpha_i*beta_i
     with alpha_i = mu_i - mean, so no per-element centering bias passes.
  5. Per-core partials summed on host; divide by (n-1)^2 + 1e-8.
"""

import numpy as np
import ml_dtypes
from contextlib import ExitStack

NCORES = 8
NTOT = 4096
DZ = 512
DN = 128
BLK = NTOT // NCORES      # 512 rows per core
MT = BLK // 128           # 4 M-tiles per core
NB = NTOT // 512          # 8 column tiles of 512
SH_Z = 1024.0             # fp16 storage shift for d2 of Z
SH_N = 256.0
HZ = 10.0                 # count-threshold half-window
HN = 2.5
KTARGET = float((NTOT * NTOT - 1) // 2 + 1)   # 8388608: lower-median rank

_BF16 = ml_dtypes.bfloat16

_nc_cache = {}


def _split_waits(nc, limit=1):
    """This walrus build accepts at most one sync-wait per instruction;
    hoist extra waits onto preceding single-wait drains on the same engine."""
    import concourse.mybir as mybir
    import bass_rust
    ctr = 0
    for f in nc.m.functions:
        for b in f.blocks:
            out, changed = [], False
            for inst in b.instructions:
                si = inst.sync_info
                waits = list(si.on_wait) if si is not None else []
                if len(waits) > limit:
                    changed = True
                    for w in waits[:-limit]:
                        ctr += 1
                        d = mybir.InstDrain(name=f"I-waitsplit-{ctr}", ins=[], outs=[])
                        d.engine = inst.engine
                        d.sync_info = bass_rust.SyncInfo(on_update=[], on_wait=[w])
                        out.append(d)
                    si.on_wait = waits[-limit:]
                out.append(inst)
            if changed:
                b.instructions = out
    return ctr


def _build():
    import concourse.bass as bass
    import concourse.mybir as mybir
    import concourse.tile as tile
    from concourse import bass_isa

    f32 = mybir.dt.float32
    f16 = mybir.dt.float16
    bf16 = mybir.dt.bfloat16
    Alu = mybir.AluOpType
    Act = mybir.ActivationFunctionType
    RG = [list(range(NCORES))]

    nc = bass.Bass("TRN2", num_devices=NCORES)

    zt = nc.dram_tensor("zt", [DZ + 2, NTOT], bf16, kind="ExternalInput")
    ntr = nc.dram_tensor("ntr", [DN + 2, NTOT], bf16, kind="ExternalInput")
    lhsz = nc.dram_tensor("lhsz", [DZ, BLK], bf16, kind="ExternalInput")
    lhsn = nc.dram_tensor("lhsn", [DN, BLK], bf16, kind="ExternalInput")
    zsqm = nc.dram_tensor("zsqm", [BLK], f32, kind="ExternalInput")   # |z_i|^2 - SH_Z
    nsqm = nc.dram_tensor("nsqm", [BLK], f32, kind="ExternalInput")   # |n_i|^2 - SH_N
    thr = nc.dram_tensor("thr", [4], f32, kind="ExternalInput")       # shifted thresholds
    out_s = nc.dram_tensor("out_s", [1, 1], f32, kind="ExternalOutput")
    out_dbg = nc.dram_tensor("out_dbg", [1, 8], f32, kind="ExternalOutput")

    KZT = DZ // 128   # 4 contraction tiles for Z
    KNT = DN // 128   # 1 for N

    with tile.TileContext(nc) as tc, ExitStack() as ctx:
        big = ctx.enter_context(tc.tile_pool(name="big", bufs=1))
        psum = ctx.enter_context(tc.tile_pool(name="psum", bufs=2, space="PSUM"))
        small = ctx.enter_context(tc.tile_pool(name="small", bufs=1))
        dram = ctx.enter_context(tc.tile_pool(name="dram", bufs=1, space="DRAM"))

        # ---------------- input DMAs (N first: its matmuls start the kernel) --
        nt_sb = big.tile([128, NTOT], bf16, tag="nk0", name="nt_sb")
        nc.sync.dma_start(nt_sb[:], ntr[0:128, :])
        ntw = small.tile([2, NTOT], bf16, tag="ntw", name="ntw")
        nc.sync.dma_start(ntw[:], ntr[DN:DN + 2, :])
        lhsn_sb = small.tile([128, BLK], bf16, tag="ln0", name="lhsn_sb")
        nc.sync.dma_start(lhsn_sb[:], lhsn[:, :])

        zt_sb = []
        for k in range(KZT):
            t = big.tile([128, NTOT], bf16, tag=f"zk{k}", name=f"zt_sb{k}")
            nc.sync.dma_start(t[:], zt[k * 128:(k + 1) * 128, :])
            zt_sb.append(t)
        ztw = small.tile([2, NTOT], bf16, tag="ztw", name="ztw")
        nc.sync.dma_start(ztw[:], zt[DZ:DZ + 2, :])
        lhsz_sb = []
        for k in range(KZT):
            t = small.tile([128, BLK], bf16, tag=f"lz{k}", name=f"lhsz_sb{k}")
            nc.sync.dma_start(t[:], lhsz[k * 128:(k + 1) * 128, :])
            lhsz_sb.append(t)

        ones2 = small.tile([2, 128], bf16, tag="ones2", name="ones2")
        nc.vector.memset(ones2[:], 1.0)

        zsqm_sb = small.tile([128, MT], f32, tag="zsqm", name="zsqm_sb")
        nc.sync.dma_start(zsqm_sb[:], zsqm[:].rearrange("(m p) -> p m", p=128))
        nsqm_sb = small.tile([128, MT], f32, tag="nsqm", name="nsqm_sb")
        nc.sync.dma_start(nsqm_sb[:], nsqm[:].rearrange("(m p) -> p m", p=128))

        thrb = small.tile([128, 4], f32, tag="thrb", name="thrb")
        thr_ap = thr[:]
        thr_b = bass.AP(tensor=thr_ap.tensor, offset=thr_ap.offset,
                        ap=[[0, 128], [1, 4]])
        nc.sync.dma_start(thrb[:], thr_b)

        ones1 = small.tile([128, 1], f32, tag="ones1", name="ones1")
        nc.vector.memset(ones1[:], 1.0)

        # ---------------- matmuls + d2s evacuation ----------------
        # d2s laid out as one [128, MT, NTOT] fp16 tile per matrix so later
        # elementwise passes are few, large ops (DVE per-op overhead ~1.5us).
        def mm_phase(d2s, lhs_tiles, rhs_tiles, wtile, sq_sb, kt, mat):
            for m in range(MT):
                ps = [psum.tile([128, 4 * 512], f32, tag="ps",
                                name=f"ps_{mat}{m}_{h}") for h in range(2)]
                for k in range(kt):
                    lw = lhs_tiles[k][:, m * 128:(m + 1) * 128]
                    for nb in range(NB):
                        nc.tensor.matmul(ps[nb // 4][:, (nb % 4) * 512:(nb % 4 + 1) * 512],
                                         lw,
                                         rhs_tiles[k][:, nb * 512:(nb + 1) * 512],
                                         start=(k == 0), stop=False)
                for nb in range(NB):
                    nc.tensor.matmul(ps[nb // 4][:, (nb % 4) * 512:(nb % 4 + 1) * 512],
                                     ones2[:, 0:128],
                                     wtile[:, nb * 512:(nb + 1) * 512],
                                     start=False, stop=True)
                for h in range(2):
                    nc.scalar.activation(d2s[:, m, h * 2048:(h + 1) * 2048],
                                         ps[h][:], Act.Identity,
                                         bias=sq_sb[:, m:m + 1], scale=-2.0)

        def count_pass(engine, d2s_m_ap, thr_ap, scr_ap, acc_ap):
            # count(d2s <= thr) over the even-column subset (x2 on host side)
            engine.tensor_scalar(scr_ap, d2s_m_ap, thr_ap, None,
                                 Alu.is_le, Alu.add, accum_out=acc_ap)

        def cdf_collective(cnt2, mat):
            # cnt2: [128, 2] per-partition counts -> global totals on all parts
            cp = psum.tile([2, 1], f32, tag="ps", name=f"cp_{mat}", bufs=None)
            nc.tensor.matmul(cp[:], cnt2, ones1[:], start=True, stop=True)
            cs = small.tile([2, 1], f32, tag=f"cs_{mat}", name=f"cs_{mat}")
            nc.scalar.activation(cs[:], cp[:], Act.Identity)
            cin = dram.tile([1, 2], f32, tag=f"cin_{mat}", name=f"cin_{mat}")
            cout = dram.tile([1, 2], f32, tag=f"cout_{mat}", name=f"cout_{mat}")
            cin_ap = cin[:]
            nc.sync.dma_start(
                bass.AP(tensor=cin_ap.tensor, offset=cin_ap.offset,
                        ap=[[1, 2], [2, 1]]), cs[:])
            nc.gpsimd.collective_compute("AllReduce", Alu.add, replica_groups=RG,
                                         ins=[cin[:]], outs=[cout[:]])
            cg = small.tile([128, 2], f32, tag=f"cg_{mat}", name=f"cg_{mat}")
            cout_ap = cout[:]
            nc.sync.dma_start(
                cg[:], bass.AP(tensor=cout_ap.tensor, offset=cout_ap.offset,
                               ap=[[0, 128], [1, 2]]))
            return cg

        scr16 = big.tile([128, MT, NTOT], f16, tag="scr", name="scr16")

        # --- N matrix first: its count->AllReduce->exp->AllGather chain
        # overlaps with the Z matmuls ---
        d2sn = big.tile([128, MT, NTOT], f16, tag="dn", name="d2sn")
        mm_phase(d2sn, [lhsn_sb], [nt_sb], ntw, nsqm_sb, KNT, "n")

        CSTRIDE = 4   # count every 4th column; rank target scales by 1/4
        nthrb = small.tile([128, 4], f32, tag="nthrb", name="nthrb")
        nc.vector.tensor_scalar(nthrb[:], thrb[:], -1.0, None, Alu.mult)

        def strided(ap3, m):
            # every 4th column of m-slice, phase m%4 so that across the four
            # m-tiles every column is sampled equally (unbiased CDF sample)
            sl = ap3[:, m, :].rearrange("p (j s) -> p j s", s=CSTRIDE)
            return sl[:, :, m % CSTRIDE]

        def counts(d2s, thr_lo_col, mat):
            # thr_lo via DVE is_le; thr_hi via ScalarE Sign (count = 2048 - sg/2)
            clo = small.tile([128, MT], f32, tag=f"clo_{mat}", name=f"clo_{mat}")
            chi = small.tile([128, MT], f32, tag=f"chi_{mat}", name=f"chi_{mat}")
            for m in range(MT):
                count_pass(nc.vector, strided(d2s, m), thrb[:, thr_lo_col:thr_lo_col + 1],
                           scr16[:, m, 0:1024], clo[:, m:m + 1])
                count_pass(nc.vector, strided(d2s, m),
                           thrb[:, thr_lo_col + 1:thr_lo_col + 2],
                           scr16[:, m, 1024:2048], chi[:, m:m + 1])
            c2 = small.tile([128, 2], f32, tag=f"c2_{mat}", name=f"c2_{mat}")
            nc.vector.tensor_reduce(c2[:, 0:1], clo[:], mybir.AxisListType.X, Alu.add)
            nc.vector.tensor_reduce(c2[:, 1:2], chi[:], mybir.AxisListType.X, Alu.add)
            return cdf_collective(c2[:], mat)

        cgn = counts(d2sn, 2, "n")

        # --- Z matrix ---
        d2sz = big.tile([128, MT, NTOT], f16, tag="dz", name="d2sz")
        mm_phase(d2sz, lhsz_sb, zt_sb, ztw, zsqm_sb, KZT, "z")

        cgz = counts(d2sz, 0, "z")

        # ---------------- median interpolation + exp coefficients ----------------
        # counts cover the even-column half of the matrix -> rank target k/2
        def interp(c0, c1, t0ap, h, shift, mat):
            num = small.tile([128, 1], f32, tag=f"num{mat}", name=f"num{mat}")
            nc.vector.tensor_scalar(num[:], c0, KTARGET / 4.0, -1.0, Alu.subtract,
                                    Alu.mult)                  # (C0-k)*-1 = k-C0
            den = small.tile([128, 1], f32, tag=f"den{mat}", name=f"den{mat}")
            nc.vector.tensor_sub(den[:], c1, c0)
            rec = small.tile([128, 1], f32, tag=f"rec{mat}", name=f"rec{mat}")
            nc.vector.reciprocal(rec[:], den[:])
            r = small.tile([128, 1], f32, tag=f"r{mat}", name=f"r{mat}")
            nc.vector.tensor_mul(r[:], num[:], rec[:])
            rc = small.tile([128, 1], f32, tag=f"rc{mat}", name=f"rc{mat}")
            nc.vector.tensor_scalar(rc[:], r[:], 0.0, 1.0, Alu.max, Alu.min)
            meds = small.tile([128, 1], f32, tag=f"meds{mat}", name=f"meds{mat}")
            nc.vector.tensor_scalar(meds[:], rc[:], 2.0 * h, t0ap, Alu.mult, Alu.add)
            tmp = small.tile([128, 1], f32, tag=f"tmp{mat}", name=f"tmp{mat}")
            nc.vector.tensor_scalar(tmp[:], meds[:], shift + 3e-8, None, Alu.add)
            s = small.tile([128, 1], f32, tag=f"s{mat}", name=f"s{mat}")
            nc.vector.reciprocal(s[:], tmp[:])
            sc = small.tile([128, 1], f32, tag=f"sc{mat}", name=f"sc{mat}")
            nc.vector.tensor_scalar(sc[:], s[:], -1.0, None, Alu.mult)
            bs = small.tile([128, 1], f32, tag=f"bs{mat}", name=f"bs{mat}")
            nc.vector.tensor_scalar(bs[:], s[:], -shift, None, Alu.mult)
            return meds, sc, bs

        medn, scn, bsn = interp(cgn[:, 0:1], cgn[:, 1:2], thrb[:, 2:3], HN, SH_N, "n")
        medz, scz, bsz = interp(cgz[:, 0:1], cgz[:, 1:2], thrb[:, 0:1], HZ, SH_Z, "z")

        # ---------------- exp (in place, d2s becomes K) + fused row sums;
        # per-matrix AllGather of row sums, broadcast to all partitions -------
        inv_n = 1.0 / NTOT
        inv_n2 = 1.0 / (NTOT * NTOT)

        def v_bcast(v):
            vf = v[:].rearrange("p c b -> p (c b)")
            return bass.AP(tensor=vf.tensor, offset=vf.offset,
                           ap=[vf.ap[0], [0, MT], vf.ap[1]])

        def exp_gather(d2s, sc, bs, mat):
            # exp in place (d2s becomes K) with fused row sums, AllGather the
            # row sums (fp16), broadcast to all partitions, then the centered
            # tile A = v_j - K in place over K, plus alpha = mu_i - mean.
            r = small.tile([128, MT], f32, tag=f"r{mat}x", name=f"r{mat}x")
            for m in range(MT):
                nc.scalar.activation(d2s[:, m, :], d2s[:, m, :], Act.Exp,
                                     bias=bs[:], scale=sc[:],
                                     accum_out=r[:, m:m + 1])
            r16 = small.tile([128, MT], f16, tag=f"r16{mat}", name=f"r16{mat}")
            nc.scalar.activation(r16[:], r[:], Act.Identity)
            agi = dram.tile([1, BLK], f16, tag=f"agi_{mat}", name=f"agi_{mat}")
            ago = dram.tile([NCORES, BLK], f16, tag=f"ago_{mat}", name=f"ago_{mat}")
            agi_ap = agi[:]
            nc.sync.dma_start(
                bass.AP(tensor=agi_ap.tensor, offset=agi_ap.offset,
                        ap=[[1, 128], [128, MT]]), r16[:])
            nc.gpsimd.collective_compute("AllGather", Alu.bypass,
                                         replica_groups=RG,
                                         ins=[agi[:]], outs=[ago[:]])
            v = big.tile([128, NCORES, BLK], f16, tag=f"v{mat}", name=f"v{mat}")
            ago_ap = ago[:]
            nc.sync.dma_start(
                v[:], bass.AP(tensor=ago_ap.tensor, offset=ago_ap.offset,
                              ap=[[0, 128], [BLK, NCORES], [1, BLK]]))
            # total of row sums -> grand mean; alpha_i = mu_i - mean
            t_ = small.tile([128, 1], f32, tag=f"t{mat}", name=f"t{mat}")
            nc.vector.tensor_reduce(t_[:], v[:], mybir.AxisListType.XY, Alu.add)
            mb = small.tile([128, 1], f32, tag=f"mb{mat}", name=f"mb{mat}")
            nc.vector.tensor_scalar(mb[:], t_[:], inv_n2, None, Alu.mult)
            al = small.tile([128, MT], f32, tag=f"al{mat}", name=f"al{mat}")
            nc.vector.tensor_scalar(al[:], r[:], inv_n, mb[:], Alu.mult,
                                    Alu.subtract)
            # centered tile (in place): d2s <- v_j/n - K
            nc.vector.scalar_tensor_tensor(d2s[:], v_bcast(v), inv_n, d2s[:],
                                           Alu.mult, Alu.subtract)
            return t_, al

        tn, an = exp_gather(d2sn, scn, bsn, "n")
        tz, az = exp_gather(d2sz, scz, bsz, "z")

        # ---------------- final product ----------------
        #   S_local = sum(A*B) - n * sum_i alpha_i beta_i
        ca = small.tile([128, MT], f32, tag="ca", name="ca")
        nc.vector.tensor_mul(ca[:], az[:], an[:])
        pacc = small.tile([128, 1], f32, tag="pacc", name="pacc")
        nc.vector.scalar_tensor_tensor(
            scr16[:], d2sz[:], 1.0, d2sn[:], Alu.mult, Alu.mult,
            accum_out=pacc[:, 0:1])

        cb = small.tile([128, 1], f32, tag="cb", name="cb")
        nc.vector.tensor_reduce(cb[:], ca[:], mybir.AxisListType.X, Alu.add)
        spp = small.tile([128, 1], f32, tag="spp", name="spp")
        nc.vector.tensor_scalar(spp[:], cb[:], -float(NTOT), pacc[:],
                                Alu.mult, Alu.add)
        stp = psum.tile([1, 1], f32, tag="ps", name="stp")
        nc.tensor.matmul(stp[:], spp[:], ones1[:], start=True, stop=True)
        stot = small.tile([1, 1], f32, tag="stot", name="stot")
        nc.scalar.activation(stot[:], stp[:], Act.Identity)
        nc.sync.dma_start(out_s[:], stot[0:1, 0:1])

        # debug outputs
        nc.sync.dma_start(out_dbg[0:1, 0:1], medz[0:1, 0:1])
        nc.sync.dma_start(out_dbg[0:1, 1:2], medn[0:1, 0:1])
        nc.sync.dma_start(out_dbg[0:1, 2:4], cgz[0:1, :])
        nc.sync.dma_start(out_dbg[0:1, 4:6], cgn[0:1, :])
        nc.sync.dma_start(out_dbg[0:1, 6:7], tz[0:1, 0:1])
        nc.sync.dma_start(out_dbg[0:1, 7:8], tn[0:1, 0:1])

    return nc


def _get_nc():
    if "nc" not in _nc_cache:
        nc = _build()
        _split_waits(nc)
        _nc_cache["nc"] = nc
    return _nc_cache["nc"]


def _sample_median(X32, xsq):
    """Host estimate of the lower-median of the pairwise squared distances."""
    rows = X32[::8]
    cols = X32[::2]
    G = rows @ cols.T
    d2 = xsq[::8, None] + xsq[None, ::2] - 2.0 * G
    flat = d2.ravel()
    return float(np.partition(flat, (flat.size - 1) // 2)[(flat.size - 1) // 2])


def _prepare_inputs(Z, N):
    Zf = np.asarray(Z, dtype=np.float32)
    Nf = np.asarray(N, dtype=np.float32)
    zsq = (Zf.astype(np.float64) ** 2).sum(1).astype(np.float32)
    nsq = (Nf.astype(np.float64) ** 2).sum(1).astype(np.float32)
    Zb = Zf.astype(_BF16)
    Nb = Nf.astype(_BF16)

    def aug(Xb, xsq):
        w = (-0.5 * xsq).astype(np.float32)
        w_hi = w.astype(_BF16)
        w_lo = (w - w_hi.astype(np.float32)).astype(_BF16)
        return np.concatenate(
            [np.ascontiguousarray(Xb.T), w_hi[None, :], w_lo[None, :]], axis=0)

    zt = aug(Zb, zsq)
    nt = aug(Nb, nsq)

    t0z = _sample_median(Zf, zsq)
    t0n = _sample_median(Nf, nsq)
    thr = np.array([t0z - HZ - SH_Z, t0z + HZ - SH_Z,
                    t0n - HN - SH_N, t0n + HN - SH_N], dtype=np.float32)
    # keep thresholds off the fp16 grid so is_le sees no exact ties
    on_grid = thr == thr.astype(np.float16).astype(np.float32)
    thr[on_grid] += np.float32(1.001953125e-3)

    in_maps = []
    for c in range(NCORES):
        sl = slice(c * BLK, (c + 1) * BLK)
        in_maps.append({
            "zt": zt,
            "ntr": nt,
            "lhsz": np.ascontiguousarray(Zb.T[:, sl]),
            "lhsn": np.ascontiguousarray(Nb.T[:, sl]),
            "zsqm": (zsq[sl] - SH_Z).astype(np.float32),
            "nsqm": (nsq[sl] - SH_N).astype(np.float32),
            "thr": thr,
        })
    return in_maps


def run_on_device(Z, N, **run_kwargs):
    """Run the bass kernel; returns (BassKernelResults, hsic float)."""
    from concourse.bass_utils import run_bass_kernel_spmd
    nc = _get_nc()
    in_maps = _prepare_inputs(Z, N)
    res = run_bass_kernel_spmd(nc, in_maps, core_ids=list(range(NCORES)),
                               **run_kwargs)
    S = sum(float(r["out_s"][0, 0]) for r in res.results)
    hsic = S / ((NTOT - 1) ** 2 + 1e-8)
    return res, hsic


def kernel(Z, N):
    _, hsic = run_on_device(Z, N)
    return np.asarray(hsic, dtype=np.float32)


if __name__ == "__main__":
    rng = np.random.default_rng(0)
    Z = rng.standard_normal((NTOT, DZ), dtype=np.float32)
    N = rng.standard_normal((NTOT, DN), dtype=np.float32)
    res, hsic = run_on_device(Z, N)
    print("hsic:", hsic)
    print("dbg core0:", res.results[0]["out_dbg"])


# revision 25
# speedup vs baseline: 1.3327x; 1.0668x over previous
"""Distributed HSIC independence loss for Trainium2 (8 NeuronCores).

Pipeline (single NEFF launch, row-sharded across 8 cores):
  1. Per core: P = Zrow @ Zfull.T via TensorE (bf16, f32 accum), with the
     -|z_j|^2/2 term folded in as two extra bf16 contraction rows (hi+lo
     split), so d2 = -2*P + |z_i|^2 comes out of PSUM in one ScalarE
     activation (stored shifted, fp16).
  2. Median of d2: host supplies a sampled estimate t0; the device computes
     exact full counts of d2 <= t0 +/- h, AllReduces the 4 counts (Z and N),
     and linearly interpolates the CDF to get the global lower-median.
  3. K = exp(-d2/(2*sigma^2+1e-8)) via one ScalarE activation per tile
     (runtime per-partition scale/bias), with fused row-sum accumulation.
  4. Row sums are AllGathered; HSIC sum computed via
     sum(Kc*Lc) = sum((v_j - K)(q_j - L)) - n * sum_i alpha_i*beta_i
     with alpha_i = mu_i - mean, so no per-element centering bias passes.
  5. Per-core partials summed on host; divide by (n-1)^2 + 1e-8.
"""

import numpy as np
import ml_dtypes
from contextlib import ExitStack

NCORES = 8
NTOT = 4096
DZ = 512
DN = 128
BLK = NTOT // NCORES      # 512 rows per core
MT = BLK // 128           # 4 M-tiles per core
NB = NTOT // 512          # 8 column tiles of 512
SH_Z = 1024.0             # fp16 storage shift for d2 of Z
SH_N = 256.0
HZ = 10.0                 # count-threshold half-window
HN = 2.5
KTARGET = float((NTOT * NTOT - 1) // 2 + 1)   # 8388608: lower-median rank

_BF16 = ml_dtypes.bfloat16

_nc_cache = {}


def _split_waits(nc, limit=1):
    """This walrus build accepts at most one sync-wait per instruction;
    hoist extra waits onto preceding single-wait drains on the same engine."""
    import concourse.mybir as mybir
    import bass_rust
    ctr = 0
    for f in nc.m.functions:
        for b in f.blocks:
            out, changed = [], False
            for inst in b.instructions:
                si = inst.sync_info
                waits = list(si.on_wait) if si is not None else []
                if len(waits) > limit:
                    changed = True
                    for w in waits[:-limit]:
                        ctr += 1
                        d = mybir.InstDrain(name=f"I-waitsplit-{ctr}", ins=[], outs=[])
                        d.engine = inst.engine
                        d.sync_info = bass_rust.SyncInfo(on_update=[], on_wait=[w])
                        out.append(d)
                    si.on_wait = waits[-limit:]
                out.append(inst)
            if changed:
                b.instructions = out
    return ctr


def _build():
    import concourse.bass as bass
    import concourse.mybir as mybir
    import concourse.tile as tile
    from concourse import bass_isa

    f32 = mybir.dt.float32
    f16 = mybir.dt.float16
    bf16 = mybir.dt.bfloat16
    Alu = mybir.AluOpType
    Act = mybir.ActivationFunctionType
    RG = [list(range(NCORES))]

    nc = bass.Bass("TRN2", num_devices=NCORES)

    zt = nc.dram_tensor("zt", [DZ + 2, NTOT], bf16, kind="ExternalInput")
    ntr = nc.dram_tensor("ntr", [DN + 2, NTOT], bf16, kind="ExternalInput")
    lhsz = nc.dram_tensor("lhsz", [DZ, BLK], bf16, kind="ExternalInput")
    lhsn = nc.dram_tensor("lhsn", [DN, BLK], bf16, kind="ExternalInput")
    zsqm = nc.dram_tensor("zsqm", [BLK], f32, kind="ExternalInput")   # |z_i|^2 - SH_Z
    nsqm = nc.dram_tensor("nsqm", [BLK], f32, kind="ExternalInput")   # |n_i|^2 - SH_N
    thr = nc.dram_tensor("thr", [4], f32, kind="ExternalInput")       # shifted thresholds
    out_s = nc.dram_tensor("out_s", [1, 1], f32, kind="ExternalOutput")
    out_dbg = nc.dram_tensor("out_dbg", [1, 8], f32, kind="ExternalOutput")

    KZT = DZ // 128   # 4 contraction tiles for Z
    KNT = DN // 128   # 1 for N

    with tile.TileContext(nc) as tc, ExitStack() as ctx:
        big = ctx.enter_context(tc.tile_pool(name="big", bufs=1))
        psum = ctx.enter_context(tc.tile_pool(name="psum", bufs=2, space="PSUM"))
        small = ctx.enter_context(tc.tile_pool(name="small", bufs=1))
        dram = ctx.enter_context(tc.tile_pool(name="dram", bufs=1, space="DRAM"))

        # ---------------- input DMAs (N first: its matmuls start the kernel) --
        nt_sb = big.tile([128, NTOT], bf16, tag="nk0", name="nt_sb")
        nc.sync.dma_start(nt_sb[:], ntr[0:128, :])
        ntw = small.tile([2, NTOT], bf16, tag="ntw", name="ntw")
        nc.sync.dma_start(ntw[:], ntr[DN:DN + 2, :])
        lhsn_sb = small.tile([128, BLK], bf16, tag="ln0", name="lhsn_sb")
        nc.sync.dma_start(lhsn_sb[:], lhsn[:, :])

        zt_sb = []
        for k in range(KZT):
            t = big.tile([128, NTOT], bf16, tag=f"zk{k}", name=f"zt_sb{k}")
            nc.sync.dma_start(t[:], zt[k * 128:(k + 1) * 128, :])
            zt_sb.append(t)
        ztw = small.tile([2, NTOT], bf16, tag="ztw", name="ztw")
        nc.sync.dma_start(ztw[:], zt[DZ:DZ + 2, :])
        lhsz_sb = []
        for k in range(KZT):
            t = small.tile([128, BLK], bf16, tag=f"lz{k}", name=f"lhsz_sb{k}")
            nc.sync.dma_start(t[:], lhsz[k * 128:(k + 1) * 128, :])
            lhsz_sb.append(t)

        ones2 = small.tile([2, 128], bf16, tag="ones2", name="ones2")
        nc.vector.memset(ones2[:], 1.0)

        zsqm_sb = small.tile([128, MT], f32, tag="zsqm", name="zsqm_sb")
        nc.sync.dma_start(zsqm_sb[:], zsqm[:].rearrange("(m p) -> p m", p=128))
        nsqm_sb = small.tile([128, MT], f32, tag="nsqm", name="nsqm_sb")
        nc.sync.dma_start(nsqm_sb[:], nsqm[:].rearrange("(m p) -> p m", p=128))

        thrb = small.tile([128, 4], f32, tag="thrb", name="thrb")
        thr_ap = thr[:]
        thr_b = bass.AP(tensor=thr_ap.tensor, offset=thr_ap.offset,
                        ap=[[0, 128], [1, 4]])
        nc.sync.dma_start(thrb[:], thr_b)

        ones1 = small.tile([128, 1], f32, tag="ones1", name="ones1")
        nc.vector.memset(ones1[:], 1.0)

        # ---------------- matmuls + d2s evacuation ----------------
        # d2s laid out as one [128, MT, NTOT] fp16 tile per matrix so later
        # elementwise passes are few, large ops (DVE per-op overhead ~1.5us).
        def mm_phase(d2s, lhs_tiles, rhs_tiles, wtile, sq_sb, kt, mat):
            for m in range(MT):
                ps = [psum.tile([128, 4 * 512], f32, tag="ps",
                                name=f"ps_{mat}{m}_{h}") for h in range(2)]
                for k in range(kt):
                    lw = lhs_tiles[k][:, m * 128:(m + 1) * 128]
                    for nb in range(NB):
                        nc.tensor.matmul(ps[nb // 4][:, (nb % 4) * 512:(nb % 4 + 1) * 512],
                                         lw,
                                         rhs_tiles[k][:, nb * 512:(nb + 1) * 512],
                                         start=(k == 0), stop=False)
                for nb in range(NB):
                    nc.tensor.matmul(ps[nb // 4][:, (nb % 4) * 512:(nb % 4 + 1) * 512],
                                     ones2[:, 0:128],
                                     wtile[:, nb * 512:(nb + 1) * 512],
                                     start=False, stop=True)
                for h in range(2):
                    nc.scalar.activation(d2s[:, m, h * 2048:(h + 1) * 2048],
                                         ps[h][:], Act.Identity,
                                         bias=sq_sb[:, m:m + 1], scale=-2.0)

        def count_pass(engine, d2s_m_ap, thr_ap, scr_ap, acc_ap):
            # count(d2s <= thr) over the even-column subset (x2 on host side)
            engine.tensor_scalar(scr_ap, d2s_m_ap, thr_ap, None,
                                 Alu.is_le, Alu.add, accum_out=acc_ap)

        def cdf_collective(cnt2, mat):
            # cnt2: [128, 2] per-partition counts -> global totals on all parts
            cp = psum.tile([2, 1], f32, tag="ps", name=f"cp_{mat}", bufs=None)
            nc.tensor.matmul(cp[:], cnt2, ones1[:], start=True, stop=True)
            cs = small.tile([2, 1], f32, tag=f"cs_{mat}", name=f"cs_{mat}")
            nc.scalar.activation(cs[:], cp[:], Act.Identity)
            cin = dram.tile([1, 2], f32, tag=f"cin_{mat}", name=f"cin_{mat}")
            cout = dram.tile([1, 2], f32, tag=f"cout_{mat}", name=f"cout_{mat}")
            cin_ap = cin[:]
            nc.sync.dma_start(
                bass.AP(tensor=cin_ap.tensor, offset=cin_ap.offset,
                        ap=[[1, 2], [2, 1]]), cs[:])
            nc.gpsimd.collective_compute("AllReduce", Alu.add, replica_groups=RG,
                                         ins=[cin[:]], outs=[cout[:]])
            cg = small.tile([128, 2], f32, tag=f"cg_{mat}", name=f"cg_{mat}")
            cout_ap = cout[:]
            nc.sync.dma_start(
                cg[:], bass.AP(tensor=cout_ap.tensor, offset=cout_ap.offset,
                               ap=[[0, 128], [1, 2]]))
            return cg

        scr16 = big.tile([128, NTOT], f16, tag="scr", name="scr16")
        scr3 = scr16[:].rearrange("p (m j) -> p m j", m=MT)

        # --- N matrix first: its count->AllReduce->exp->AllGather chain
        # overlaps with the Z matmuls ---
        d2sn = big.tile([128, MT, NTOT], f16, tag="dn", name="d2sn")
        mm_phase(d2sn, [lhsn_sb], [nt_sb], ntw, nsqm_sb, KNT, "n")

        CSTRIDE = 4   # count every 4th column; rank target scales by 1/4

        def strided(ap3, m):
            # every 4th column of m-slice, phase m%4 so that across the four
            # m-tiles every column is sampled equally (unbiased CDF sample)
            sl = ap3[:, m, :].rearrange("p (j s) -> p j s", s=CSTRIDE)
            return sl[:, :, m % CSTRIDE]

        def counts(d2s, thr_lo_col, mat):
            # thr_lo via DVE is_le; thr_hi via ScalarE Sign (count = 2048 - sg/2)
            clo = small.tile([128, MT], f32, tag=f"clo_{mat}", name=f"clo_{mat}")
            chi = small.tile([128, MT], f32, tag=f"chi_{mat}", name=f"chi_{mat}")
            for m in range(MT):
                count_pass(nc.vector, strided(d2s, m), thrb[:, thr_lo_col:thr_lo_col + 1],
                           scr3[:, m, 0:1024], clo[:, m:m + 1])
                count_pass(nc.vector, strided(d2s, m),
                           thrb[:, thr_lo_col + 1:thr_lo_col + 2],
                           scr3[:, m, 0:1024], chi[:, m:m + 1])
            c2 = small.tile([128, 2], f32, tag=f"c2_{mat}", name=f"c2_{mat}")
            nc.vector.tensor_reduce(c2[:, 0:1], clo[:], mybir.AxisListType.X, Alu.add)
            nc.vector.tensor_reduce(c2[:, 1:2], chi[:], mybir.AxisListType.X, Alu.add)
            return cdf_collective(c2[:], mat)

        cgn = counts(d2sn, 2, "n")

        # --- Z matrix ---
        d2sz = big.tile([128, MT, NTOT], f16, tag="dz", name="d2sz")
        mm_phase(d2sz, lhsz_sb, zt_sb, ztw, zsqm_sb, KZT, "z")

        cgz = counts(d2sz, 0, "z")

        # ---------------- median interpolation + exp coefficients ----------------
        # counts cover the even-column half of the matrix -> rank target k/2
        def interp(c0, c1, t0ap, h, shift, mat):
            num = small.tile([128, 1], f32, tag=f"num{mat}", name=f"num{mat}")
            nc.vector.tensor_scalar(num[:], c0, KTARGET / 4.0, -1.0, Alu.subtract,
                                    Alu.mult)                  # (C0-k)*-1 = k-C0
            den = small.tile([128, 1], f32, tag=f"den{mat}", name=f"den{mat}")
            nc.vector.tensor_sub(den[:], c1, c0)
            rec = small.tile([128, 1], f32, tag=f"rec{mat}", name=f"rec{mat}")
            nc.vector.reciprocal(rec[:], den[:])
            r = small.tile([128, 1], f32, tag=f"r{mat}", name=f"r{mat}")
            nc.vector.tensor_mul(r[:], num[:], rec[:])
            rc = small.tile([128, 1], f32, tag=f"rc{mat}", name=f"rc{mat}")
            nc.vector.tensor_scalar(rc[:], r[:], 0.0, 1.0, Alu.max, Alu.min)
            meds = small.tile([128, 1], f32, tag=f"meds{mat}", name=f"meds{mat}")
            nc.vector.tensor_scalar(meds[:], rc[:], 2.0 * h, t0ap, Alu.mult, Alu.add)
            tmp = small.tile([128, 1], f32, tag=f"tmp{mat}", name=f"tmp{mat}")
            nc.vector.tensor_scalar(tmp[:], meds[:], shift + 3e-8, None, Alu.add)
            s = small.tile([128, 1], f32, tag=f"s{mat}", name=f"s{mat}")
            nc.vector.reciprocal(s[:], tmp[:])
            sc = small.tile([128, 1], f32, tag=f"sc{mat}", name=f"sc{mat}")
            nc.vector.tensor_scalar(sc[:], s[:], -1.0, None, Alu.mult)
            bs = small.tile([128, 1], f32, tag=f"bs{mat}", name=f"bs{mat}")
            nc.vector.tensor_scalar(bs[:], s[:], -shift, None, Alu.mult)
            return meds, sc, bs

        medn, scn, bsn = interp(cgn[:, 0:1], cgn[:, 1:2], thrb[:, 2:3], HN, SH_N, "n")
        medz, scz, bsz = interp(cgz[:, 0:1], cgz[:, 1:2], thrb[:, 0:1], HZ, SH_Z, "z")

        # ---------------- exp (in place, d2s becomes K) + fused row sums;
        # per-matrix AllGather of row sums, broadcast to all partitions -------
        inv_n = 1.0 / NTOT
        inv_n2 = 1.0 / (NTOT * NTOT)

        def v_bcast(v):
            vf = v[:].rearrange("p c b -> p (c b)")
            return bass.AP(tensor=vf.tensor, offset=vf.offset,
                           ap=[vf.ap[0], [0, MT], vf.ap[1]])

        def exp_rows(d2s, sc, bs, mat):
            # exp in place (d2s becomes K) with fused row sums; AllGather the
            # row sums (fp16).
            r = small.tile([128, MT], f32, tag=f"r{mat}x", name=f"r{mat}x")
            for m in range(MT):
                nc.scalar.activation(d2s[:, m, :], d2s[:, m, :], Act.Exp,
                                     bias=bs[:], scale=sc[:],
                                     accum_out=r[:, m:m + 1])
            r16 = small.tile([128, MT], f16, tag=f"r16{mat}", name=f"r16{mat}")
            nc.scalar.activation(r16[:], r[:], Act.Identity)
            agi = dram.tile([1, BLK], f16, tag=f"agi_{mat}", name=f"agi_{mat}")
            ago = dram.tile([NCORES, BLK], f16, tag=f"ago_{mat}", name=f"ago_{mat}")
            agi_ap = agi[:]
            nc.sync.dma_start(
                bass.AP(tensor=agi_ap.tensor, offset=agi_ap.offset,
                        ap=[[1, 128], [128, MT]]), r16[:])
            nc.gpsimd.collective_compute("AllGather", Alu.bypass,
                                         replica_groups=RG,
                                         ins=[agi[:]], outs=[ago[:]])
            return r, ago

        def row_of(ago, parts):
            # [parts, NTOT] view of the gathered row sums (partition-bcast)
            ago_ap = ago[:]
            return bass.AP(tensor=ago_ap.tensor, offset=ago_ap.offset,
                           ap=[[0, parts], [BLK, NCORES], [1, BLK]])

        rn, agon = exp_rows(d2sn, scn, bsn, "n")
        # B = q_j - L in place over L = d2sn, q_j = R^L_j / n (bcast tile)
        vn = big.tile([128, NCORES, BLK], f16, tag="vn", name="vn")
        nc.sync.dma_start(vn[:], row_of(agon, 128))
        nc.vector.scalar_tensor_tensor(d2sn[:], v_bcast(vn), inv_n, d2sn[:],
                                       Alu.mult, Alu.subtract)

        rz, agoz = exp_rows(d2sz, scz, bsz, "z")
        vzrow = small.tile([1, NTOT], f16, tag="vzrow", name="vzrow")
        nc.sync.dma_start(vzrow[:], row_of(agoz, 1))
        vnrow = small.tile([1, NTOT], f16, tag="vnrow", name="vnrow")
        nc.sync.dma_start(vnrow[:], row_of(agon, 1))

        # ---------------- final assembly ----------------
        # S_core = dotZ/n - sum(K.B) - P1/n + mbL*P2 + mbK*P3 - 512*n*mbK*mbL
        # where B = q_j - L, dotZ = sum_j R^K_j * colB_j (colB over local rows),
        # P1 = sum_i R^K_i R^L_i, P2 = sum_i R^K_i, P3 = sum_i R^L_i (local i),
        # mbK/mbL the grand means of K/L.
        # sum(K.B): one fused pass, per-partition accum
        kb4 = small.tile([128, MT], f32, tag="kb4", name="kb4")
        for m in range(MT):
            nc.vector.scalar_tensor_tensor(
                scr16[:], d2sz[:, m, :], 1.0, d2sn[:, m, :], Alu.mult, Alu.mult,
                accum_out=kb4[:, m:m + 1])

        # colB via ones-matmuls on PE (B is fp16)
        ones1h = small.tile([128, 1], f16, tag="ones1h", name="ones1h")
        nc.vector.memset(ones1h[:], 1.0)
        colb = small.tile([1, NTOT], f32, tag="colb", name="colb")
        for h in range(2):
            pc = psum.tile([1, 2048], f32, tag="ps", name=f"pcolb{h}")
            for q in range(4):
                cslice = slice(h * 2048 + q * 512, h * 2048 + (q + 1) * 512)
                for m in range(MT):
                    nc.tensor.matmul(pc[:, q * 512:(q + 1) * 512], ones1h[:],
                                     d2sn[:, m, cslice],
                                     start=(m == 0), stop=(m == MT - 1))
            nc.scalar.activation(colb[:, h * 2048:(h + 1) * 2048], pc[:],
                                 Act.Identity)

        # per-partition pieces -> one [1,4] partition-sum matmul
        u1 = small.tile([128, 1], f32, tag="u1", name="u1")
        nc.vector.scalar_tensor_tensor(scr16[:, 0:MT], rz[:], 1.0, rn[:],
                                       Alu.mult, Alu.mult, accum_out=u1[:, 0:1])
        u2 = small.tile([128, 1], f32, tag="u2", name="u2")
        nc.vector.tensor_reduce(u2[:], rz[:], mybir.AxisListType.X, Alu.add)
        u3 = small.tile([128, 1], f32, tag="u3", name="u3")
        nc.vector.tensor_reduce(u3[:], rn[:], mybir.AxisListType.X, Alu.add)
        wq = small.tile([128, 4], f32, tag="wq", name="wq")
        nc.vector.tensor_copy(wq[:, 0:1], u1[:])
        nc.vector.tensor_copy(wq[:, 1:2], u2[:])
        nc.vector.tensor_copy(wq[:, 2:3], u3[:])
        nc.vector.tensor_reduce(wq[:, 3:4], kb4[:], mybir.AxisListType.X, Alu.add)
        wp = psum.tile([1, 4], f32, tag="ps", name="wp")
        nc.tensor.matmul(wp[:], ones1[:], wq[:], start=True, stop=True)
        ws = small.tile([1, 4], f32, tag="ws", name="ws")
        nc.scalar.activation(ws[:], wp[:], Act.Identity)

        # dotZ = sum_j R^K_j * colB_j  (single-partition fused pass)
        scrow = small.tile([1, NTOT], f16, tag="scrow", name="scrow")
        dz1 = small.tile([1, 1], f32, tag="dz1", name="dz1")
        nc.vector.scalar_tensor_tensor(scrow[:], colb[:], inv_n, vzrow[:],
                                       Alu.mult, Alu.mult, accum_out=dz1[:, 0:1])
        # grand means
        tkr = small.tile([1, 1], f32, tag="tkr", name="tkr")
        nc.vector.tensor_reduce(tkr[:], vzrow[:], mybir.AxisListType.X, Alu.add)
        tlr = small.tile([1, 1], f32, tag="tlr", name="tlr")
        nc.vector.tensor_reduce(tlr[:], vnrow[:], mybir.AxisListType.X, Alu.add)
        mbk = small.tile([1, 1], f32, tag="mbk", name="mbk")
        nc.vector.tensor_scalar(mbk[:], tkr[:], inv_n2, None, Alu.mult)
        mbl = small.tile([1, 1], f32, tag="mbl", name="mbl")
        nc.vector.tensor_scalar(mbl[:], tlr[:], inv_n2, None, Alu.mult)

        # combine: S = dz1 - P1/n - KB + mbl*P2 + mbk*P3 - 512*n*mbk*mbl
        e1 = small.tile([1, 1], f32, tag="e1", name="e1")
        nc.vector.tensor_scalar(e1[:], ws[0:1, 0:1], inv_n, dz1[:],
                                Alu.mult, Alu.subtract)      # P1/n - dz1
        e2 = small.tile([1, 1], f32, tag="e2", name="e2")
        nc.vector.tensor_add(e2[:], e1[:], ws[0:1, 3:4])     # P1/n - dz1 + KB
        e3 = small.tile([1, 1], f32, tag="e3", name="e3")
        nc.vector.tensor_mul(e3[:], mbl[:], ws[0:1, 1:2])    # mbl*P2
        e4 = small.tile([1, 1], f32, tag="e4", name="e4")
        nc.vector.tensor_mul(e4[:], mbk[:], ws[0:1, 2:3])    # mbk*P3
        e5 = small.tile([1, 1], f32, tag="e5", name="e5")
        nc.vector.tensor_mul(e5[:], mbk[:], mbl[:])
        e6 = small.tile([1, 1], f32, tag="e6", name="e6")
        nc.vector.tensor_scalar(e6[:], e5[:], -float(BLK * NTOT), None, Alu.mult)
        e7 = small.tile([1, 1], f32, tag="e7", name="e7")
        nc.vector.tensor_add(e7[:], e3[:], e4[:])
        e8 = small.tile([1, 1], f32, tag="e8", name="e8")
        nc.vector.tensor_add(e8[:], e7[:], e6[:])
        sfin = small.tile([1, 1], f32, tag="sfin", name="sfin")
        nc.vector.tensor_sub(sfin[:], e8[:], e2[:])
        nc.sync.dma_start(out_s[:], sfin[0:1, 0:1])

        # debug outputs
        nc.sync.dma_start(out_dbg[0:1, 0:1], medz[0:1, 0:1])
        nc.sync.dma_start(out_dbg[0:1, 1:2], medn[0:1, 0:1])
        nc.sync.dma_start(out_dbg[0:1, 2:4], cgz[0:1, :])
        nc.sync.dma_start(out_dbg[0:1, 4:6], cgn[0:1, :])
        nc.sync.dma_start(out_dbg[0:1, 6:7], tkr[0:1, 0:1])
        nc.sync.dma_start(out_dbg[0:1, 7:8], tlr[0:1, 0:1])

    return nc


def _get_nc():
    if "nc" not in _nc_cache:
        nc = _build()
        _split_waits(nc)
        _nc_cache["nc"] = nc
    return _nc_cache["nc"]


def _sample_median(X32, xsq):
    """Host estimate of the lower-median of the pairwise squared distances."""
    rows = X32[::8]
    cols = X32[::2]
    G = rows @ cols.T
    d2 = xsq[::8, None] + xsq[None, ::2] - 2.0 * G
    flat = d2.ravel()
    return float(np.partition(flat, (flat.size - 1) // 2)[(flat.size - 1) // 2])


def _prepare_inputs(Z, N):
    Zf = np.asarray(Z, dtype=np.float32)
    Nf = np.asarray(N, dtype=np.float32)
    zsq = (Zf.astype(np.float64) ** 2).sum(1).astype(np.float32)
    nsq = (Nf.astype(np.float64) ** 2).sum(1).astype(np.float32)
    Zb = Zf.astype(_BF16)
    Nb = Nf.astype(_BF16)

    def aug(Xb, xsq):
        w = (-0.5 * xsq).astype(np.float32)
        w_hi = w.astype(_BF16)
        w_lo = (w - w_hi.astype(np.float32)).astype(_BF16)
        return np.concatenate(
            [np.ascontiguousarray(Xb.T), w_hi[None, :], w_lo[None, :]], axis=0)

    zt = aug(Zb, zsq)
    nt = aug(Nb, nsq)

    t0z = _sample_median(Zf, zsq)
    t0n = _sample_median(Nf, nsq)
    thr = np.array([t0z - HZ - SH_Z, t0z + HZ - SH_Z,
                    t0n - HN - SH_N, t0n + HN - SH_N], dtype=np.float32)
    # keep thresholds off the fp16 grid so is_le sees no exact ties
    on_grid = thr == thr.astype(np.float16).astype(np.float32)
    thr[on_grid] += np.float32(1.001953125e-3)

    in_maps = []
    for c in range(NCORES):
        sl = slice(c * BLK, (c + 1) * BLK)
        in_maps.append({
            "zt": zt,
            "ntr": nt,
            "lhsz": np.ascontiguousarray(Zb.T[:, sl]),
            "lhsn": np.ascontiguousarray(Nb.T[:, sl]),
            "zsqm": (zsq[sl] - SH_Z).astype(np.float32),
            "nsqm": (nsq[sl] - SH_N).astype(np.float32),
            "thr": thr,
        })
    return in_maps


def run_on_device(Z, N, **run_kwargs):
    """Run the bass kernel; returns (BassKernelResults, hsic float)."""
    from concourse.bass_utils import run_bass_kernel_spmd
    nc = _get_nc()
    in_maps = _prepare_inputs(Z, N)
    res = run_bass_kernel_spmd(nc, in_maps, core_ids=list(range(NCORES)),
                               **run_kwargs)
    S = sum(float(r["out_s"][0, 0]) for r in res.results)
    hsic = S / ((NTOT - 1) ** 2 + 1e-8)
    return res, hsic


def kernel(Z, N):
    _, hsic = run_on_device(Z, N)
    return np.asarray(hsic, dtype=np.float32)


if __name__ == "__main__":
    rng = np.random.default_rng(0)
    Z = rng.standard_normal((NTOT, DZ), dtype=np.float32)
    N = rng.standard_normal((NTOT, DN), dtype=np.float32)
    res, hsic = run_on_device(Z, N)
    print("hsic:", hsic)
    print("dbg core0:", res.results[0]["out_dbg"])


# revision 26
# speedup vs baseline: 1.5076x; 1.1313x over previous
"""Distributed HSIC independence loss for Trainium2 (8 NeuronCores).

Pipeline (single NEFF launch, row-sharded across 8 cores):
  1. Per core: P = Zrow @ Zfull.T via TensorE (bf16, f32 accum), with the
     -|z_j|^2/2 term folded in as two extra bf16 contraction rows (hi+lo
     split), so d2 = -2*P + |z_i|^2 comes out of PSUM in one ScalarE
     activation (stored shifted, fp16).
  2. Median of d2: host supplies a sampled estimate t0; the device computes
     exact full counts of d2 <= t0 +/- h, AllReduces the 4 counts (Z and N),
     and linearly interpolates the CDF to get the global lower-median.
  3. K = exp(-d2/(2*sigma^2+1e-8)) via one ScalarE activation per tile
     (runtime per-partition scale/bias), with fused row-sum accumulation.
  4. Row sums are AllGathered; HSIC sum computed via
     sum(Kc*Lc) = sum((v_j - K)(q_j - L)) - n * sum_i alpha_i*beta_i
     with alpha_i = mu_i - mean, so no per-element centering bias passes.
  5. Per-core partials summed on host; divide by (n-1)^2 + 1e-8.
"""

import numpy as np
import ml_dtypes
from contextlib import ExitStack

NCORES = 8
NTOT = 4096
DZ = 512
DN = 128
BLK = NTOT // NCORES      # 512 rows per core
MT = BLK // 128           # 4 M-tiles per core
NB = NTOT // 512          # 8 column tiles of 512
SH_Z = 1024.0             # fp16 storage shift for d2 of Z
SH_N = 256.0
HZ = 10.0                 # count-threshold half-window
HN = 2.5
KTARGET = float((NTOT * NTOT - 1) // 2 + 1)   # 8388608: lower-median rank

_BF16 = ml_dtypes.bfloat16

_nc_cache = {}


def _split_waits(nc, limit=1):
    """This walrus build accepts at most one sync-wait per instruction;
    hoist extra waits onto preceding single-wait drains on the same engine."""
    import concourse.mybir as mybir
    import bass_rust
    ctr = 0
    for f in nc.m.functions:
        for b in f.blocks:
            out, changed = [], False
            for inst in b.instructions:
                si = inst.sync_info
                waits = list(si.on_wait) if si is not None else []
                if len(waits) > limit:
                    changed = True
                    for w in waits[:-limit]:
                        ctr += 1
                        d = mybir.InstDrain(name=f"I-waitsplit-{ctr}", ins=[], outs=[])
                        d.engine = inst.engine
                        d.sync_info = bass_rust.SyncInfo(on_update=[], on_wait=[w])
                        out.append(d)
                    si.on_wait = waits[-limit:]
                out.append(inst)
            if changed:
                b.instructions = out
    return ctr


def _build():
    import concourse.bass as bass
    import concourse.mybir as mybir
    import concourse.tile as tile
    from concourse import bass_isa

    f32 = mybir.dt.float32
    f16 = mybir.dt.float16
    bf16 = mybir.dt.bfloat16
    Alu = mybir.AluOpType
    Act = mybir.ActivationFunctionType
    RG = [list(range(NCORES))]

    nc = bass.Bass("TRN2", num_devices=NCORES)

    zt = nc.dram_tensor("zt", [DZ + 2, NTOT], bf16, kind="ExternalInput")
    ntr = nc.dram_tensor("ntr", [DN + 2, NTOT], bf16, kind="ExternalInput")
    lhsz = nc.dram_tensor("lhsz", [DZ, BLK], bf16, kind="ExternalInput")
    lhsn = nc.dram_tensor("lhsn", [DN, BLK], bf16, kind="ExternalInput")
    zsqm = nc.dram_tensor("zsqm", [BLK], f32, kind="ExternalInput")   # |z_i|^2 - SH_Z
    nsqm = nc.dram_tensor("nsqm", [BLK], f32, kind="ExternalInput")   # |n_i|^2 - SH_N
    thr = nc.dram_tensor("thr", [4], f32, kind="ExternalInput")       # shifted thresholds
    out_s = nc.dram_tensor("out_s", [1, 1], f32, kind="ExternalOutput")
    out_dbg = nc.dram_tensor("out_dbg", [1, 8], f32, kind="ExternalOutput")

    KZT = DZ // 128   # 4 contraction tiles for Z
    KNT = DN // 128   # 1 for N

    with tile.TileContext(nc) as tc, ExitStack() as ctx:
        big = ctx.enter_context(tc.tile_pool(name="big", bufs=1))
        psum = ctx.enter_context(tc.tile_pool(name="psum", bufs=2, space="PSUM"))
        small = ctx.enter_context(tc.tile_pool(name="small", bufs=1))
        dram = ctx.enter_context(tc.tile_pool(name="dram", bufs=1, space="DRAM"))

        # ---------------- input DMAs (N first: its matmuls start the kernel) --
        nt_sb = big.tile([128, NTOT], bf16, tag="nk0", name="nt_sb")
        nc.sync.dma_start(nt_sb[:], ntr[0:128, :])
        ntw = small.tile([2, NTOT], bf16, tag="ntw", name="ntw")
        nc.sync.dma_start(ntw[:], ntr[DN:DN + 2, :])
        lhsn_sb = small.tile([128, BLK], bf16, tag="ln0", name="lhsn_sb")
        nc.sync.dma_start(lhsn_sb[:], lhsn[:, :])

        zt_sb = []
        for k in range(KZT):
            t = big.tile([128, NTOT], bf16, tag=f"zk{k}", name=f"zt_sb{k}")
            nc.sync.dma_start(t[:], zt[k * 128:(k + 1) * 128, :])
            zt_sb.append(t)
        ztw = small.tile([2, NTOT], bf16, tag="ztw", name="ztw")
        nc.sync.dma_start(ztw[:], zt[DZ:DZ + 2, :])
        lhsz_sb = []
        for k in range(KZT):
            t = small.tile([128, BLK], bf16, tag=f"lz{k}", name=f"lhsz_sb{k}")
            nc.sync.dma_start(t[:], lhsz[k * 128:(k + 1) * 128, :])
            lhsz_sb.append(t)

        ones2 = small.tile([2, 128], bf16, tag="ones2", name="ones2")
        nc.vector.memset(ones2[:], 1.0)

        zsqm_sb = small.tile([128, MT], f32, tag="zsqm", name="zsqm_sb")
        nc.sync.dma_start(zsqm_sb[:], zsqm[:].rearrange("(m p) -> p m", p=128))
        nsqm_sb = small.tile([128, MT], f32, tag="nsqm", name="nsqm_sb")
        nc.sync.dma_start(nsqm_sb[:], nsqm[:].rearrange("(m p) -> p m", p=128))

        thrb = small.tile([128, 4], f32, tag="thrb", name="thrb")
        thr_ap = thr[:]
        thr_b = bass.AP(tensor=thr_ap.tensor, offset=thr_ap.offset,
                        ap=[[0, 128], [1, 4]])
        nc.sync.dma_start(thrb[:], thr_b)

        ones1 = small.tile([128, 1], f32, tag="ones1", name="ones1")
        nc.vector.memset(ones1[:], 1.0)

        # ---------------- matmuls + d2s evacuation ----------------
        # d2s laid out as one [128, MT, NTOT] fp16 tile per matrix so later
        # elementwise passes are few, large ops (DVE per-op overhead ~1.5us).
        def mm_phase(d2s, lhs_tiles, rhs_tiles, wtile, sq_sb, kt, mat):
            for m in range(MT):
                ps = [psum.tile([128, 4 * 512], f32, tag="ps",
                                name=f"ps_{mat}{m}_{h}") for h in range(2)]
                for k in range(kt):
                    lw = lhs_tiles[k][:, m * 128:(m + 1) * 128]
                    for nb in range(NB):
                        nc.tensor.matmul(ps[nb // 4][:, (nb % 4) * 512:(nb % 4 + 1) * 512],
                                         lw,
                                         rhs_tiles[k][:, nb * 512:(nb + 1) * 512],
                                         start=(k == 0), stop=False)
                for nb in range(NB):
                    nc.tensor.matmul(ps[nb // 4][:, (nb % 4) * 512:(nb % 4 + 1) * 512],
                                     ones2[:, 0:128],
                                     wtile[:, nb * 512:(nb + 1) * 512],
                                     start=False, stop=True)
                for h in range(2):
                    if mat == "z" and m >= 2:
                        nc.vector.tensor_scalar(
                            d2s[:, m, h * 2048:(h + 1) * 2048], ps[h][:],
                            -2.0, sq_sb[:, m:m + 1], Alu.mult, Alu.add)
                    else:
                        nc.scalar.activation(d2s[:, m, h * 2048:(h + 1) * 2048],
                                             ps[h][:], Act.Identity,
                                             bias=sq_sb[:, m:m + 1], scale=-2.0)

        def count_pass(engine, d2s_m_ap, thr_ap, scr_ap, acc_ap):
            # count(d2s <= thr) over the even-column subset (x2 on host side)
            engine.tensor_scalar(scr_ap, d2s_m_ap, thr_ap, None,
                                 Alu.is_le, Alu.add, accum_out=acc_ap)

        def cdf_collective(cnt2, mat):
            # cnt2: [128, 2] per-partition counts -> global totals on all parts
            cp = psum.tile([2, 1], f32, tag="ps", name=f"cp_{mat}", bufs=None)
            nc.tensor.matmul(cp[:], cnt2, ones1[:], start=True, stop=True)
            cs = small.tile([2, 1], f32, tag=f"cs_{mat}", name=f"cs_{mat}")
            nc.scalar.activation(cs[:], cp[:], Act.Identity)
            cin = dram.tile([1, 2], f32, tag=f"cin_{mat}", name=f"cin_{mat}")
            cout = dram.tile([1, 2], f32, tag=f"cout_{mat}", name=f"cout_{mat}")
            cin_ap = cin[:]
            nc.gpsimd.dma_start(
                bass.AP(tensor=cin_ap.tensor, offset=cin_ap.offset,
                        ap=[[1, 2], [2, 1]]), cs[:])
            nc.gpsimd.collective_compute("AllReduce", Alu.add, replica_groups=RG,
                                         ins=[cin[:]], outs=[cout[:]])
            cg = small.tile([128, 2], f32, tag=f"cg_{mat}", name=f"cg_{mat}")
            cout_ap = cout[:]
            nc.sync.dma_start(
                cg[:], bass.AP(tensor=cout_ap.tensor, offset=cout_ap.offset,
                               ap=[[0, 128], [1, 2]]))
            return cg

        scr16 = big.tile([128, NTOT], f16, tag="scr", name="scr16")
        scr3 = scr16[:].rearrange("p (m j) -> p m j", m=MT)

        # --- N matrix first: its count->AllReduce->exp->AllGather chain
        # overlaps with the Z matmuls ---
        d2sn = big.tile([128, MT, NTOT], f16, tag="dn", name="d2sn")
        mm_phase(d2sn, [lhsn_sb], [nt_sb], ntw, nsqm_sb, KNT, "n")

        CSTRIDE = 4   # count every 4th column; rank target scales by 1/4

        def strided(ap3, m):
            # every 4th column of m-slice, phase m%4 so that across the four
            # m-tiles every column is sampled equally (unbiased CDF sample)
            sl = ap3[:, m, :].rearrange("p (j s) -> p j s", s=CSTRIDE)
            return sl[:, :, m % CSTRIDE]

        def counts(d2s, thr_lo_col, mat):
            # thr_lo via DVE is_le; thr_hi via ScalarE Sign (count = 2048 - sg/2)
            clo = small.tile([128, MT], f32, tag=f"clo_{mat}", name=f"clo_{mat}")
            chi = small.tile([128, MT], f32, tag=f"chi_{mat}", name=f"chi_{mat}")
            for m in range(MT):
                count_pass(nc.vector, strided(d2s, m), thrb[:, thr_lo_col:thr_lo_col + 1],
                           scr3[:, m, 0:1024], clo[:, m:m + 1])
                count_pass(nc.vector, strided(d2s, m),
                           thrb[:, thr_lo_col + 1:thr_lo_col + 2],
                           scr3[:, m, 0:1024], chi[:, m:m + 1])
            c2 = small.tile([128, 2], f32, tag=f"c2_{mat}", name=f"c2_{mat}")
            nc.vector.tensor_reduce(c2[:, 0:1], clo[:], mybir.AxisListType.X, Alu.add)
            nc.vector.tensor_reduce(c2[:, 1:2], chi[:], mybir.AxisListType.X, Alu.add)
            return cdf_collective(c2[:], mat)

        cgn = counts(d2sn, 2, "n")

        # --- Z matrix ---
        d2sz = big.tile([128, MT, NTOT], f16, tag="dz", name="d2sz")
        mm_phase(d2sz, lhsz_sb, zt_sb, ztw, zsqm_sb, KZT, "z")

        cgz = counts(d2sz, 0, "z")

        # ---------------- median interpolation + exp coefficients ----------------
        # counts cover the even-column half of the matrix -> rank target k/2
        def interp(c0, c1, t0ap, h, shift, mat):
            num = small.tile([128, 1], f32, tag=f"num{mat}", name=f"num{mat}")
            nc.vector.tensor_scalar(num[:], c0, KTARGET / 4.0, -1.0, Alu.subtract,
                                    Alu.mult)                  # (C0-k)*-1 = k-C0
            den = small.tile([128, 1], f32, tag=f"den{mat}", name=f"den{mat}")
            nc.vector.tensor_sub(den[:], c1, c0)
            rec = small.tile([128, 1], f32, tag=f"rec{mat}", name=f"rec{mat}")
            nc.vector.reciprocal(rec[:], den[:])
            r = small.tile([128, 1], f32, tag=f"r{mat}", name=f"r{mat}")
            nc.vector.tensor_mul(r[:], num[:], rec[:])
            rc = small.tile([128, 1], f32, tag=f"rc{mat}", name=f"rc{mat}")
            nc.vector.tensor_scalar(rc[:], r[:], 0.0, 1.0, Alu.max, Alu.min)
            meds = small.tile([128, 1], f32, tag=f"meds{mat}", name=f"meds{mat}")
            nc.vector.tensor_scalar(meds[:], rc[:], 2.0 * h, t0ap, Alu.mult, Alu.add)
            tmp = small.tile([128, 1], f32, tag=f"tmp{mat}", name=f"tmp{mat}")
            nc.vector.tensor_scalar(tmp[:], meds[:], shift + 3e-8, None, Alu.add)
            s = small.tile([128, 1], f32, tag=f"s{mat}", name=f"s{mat}")
            nc.vector.reciprocal(s[:], tmp[:])
            sc = small.tile([128, 1], f32, tag=f"sc{mat}", name=f"sc{mat}")
            nc.vector.tensor_scalar(sc[:], s[:], -1.0, None, Alu.mult)
            bs = small.tile([128, 1], f32, tag=f"bs{mat}", name=f"bs{mat}")
            nc.vector.tensor_scalar(bs[:], s[:], -shift, None, Alu.mult)
            return meds, sc, bs

        medn, scn, bsn = interp(cgn[:, 0:1], cgn[:, 1:2], thrb[:, 2:3], HN, SH_N, "n")
        medz, scz, bsz = interp(cgz[:, 0:1], cgz[:, 1:2], thrb[:, 0:1], HZ, SH_Z, "z")

        # ---------------- exp (in place, d2s becomes K) + fused row sums;
        # per-matrix AllGather of row sums, broadcast to all partitions -------
        inv_n = 1.0 / NTOT
        inv_n2 = 1.0 / (NTOT * NTOT)

        def v_bcast(v):
            vf = v[:].rearrange("p c b -> p (c b)")
            return bass.AP(tensor=vf.tensor, offset=vf.offset,
                           ap=[vf.ap[0], [0, MT], vf.ap[1]])

        def exp_rows(d2s, sc, bs, mat):
            # exp in place (d2s becomes K) with fused row sums; AllGather the
            # row sums (fp16).
            r = small.tile([128, MT], f32, tag=f"r{mat}x", name=f"r{mat}x")
            for m in range(MT):
                nc.scalar.activation(d2s[:, m, :], d2s[:, m, :], Act.Exp,
                                     bias=bs[:], scale=sc[:],
                                     accum_out=r[:, m:m + 1])
            r16 = small.tile([128, MT], f16, tag=f"r16{mat}", name=f"r16{mat}")
            nc.scalar.activation(r16[:], r[:], Act.Identity)
            agi = dram.tile([1, BLK], f16, tag=f"agi_{mat}", name=f"agi_{mat}")
            ago = dram.tile([NCORES, BLK], f16, tag=f"ago_{mat}", name=f"ago_{mat}")
            agi_ap = agi[:]
            nc.gpsimd.dma_start(
                bass.AP(tensor=agi_ap.tensor, offset=agi_ap.offset,
                        ap=[[1, 128], [128, MT]]), r16[:])
            nc.gpsimd.collective_compute("AllGather", Alu.bypass,
                                         replica_groups=RG,
                                         ins=[agi[:]], outs=[ago[:]])
            return r, ago

        def row_of(ago, parts):
            # [parts, NTOT] view of the gathered row sums (partition-bcast)
            ago_ap = ago[:]
            return bass.AP(tensor=ago_ap.tensor, offset=ago_ap.offset,
                           ap=[[0, parts], [BLK, NCORES], [1, BLK]])

        rn, agon = exp_rows(d2sn, scn, bsn, "n")
        # B = q_j - L in place over L = d2sn, q_j = R^L_j / n (bcast tile)
        vn = big.tile([128, NCORES, BLK], f16, tag="vn", name="vn")
        nc.sync.dma_start(vn[:], row_of(agon, 128))
        nc.vector.scalar_tensor_tensor(d2sn[:], v_bcast(vn), inv_n, d2sn[:],
                                       Alu.mult, Alu.subtract)

        rz, agoz = exp_rows(d2sz, scz, bsz, "z")
        vzrow = small.tile([1, NTOT], f16, tag="vzrow", name="vzrow")
        nc.sync.dma_start(vzrow[:], row_of(agoz, 1))
        vnrow = small.tile([1, NTOT], f16, tag="vnrow", name="vnrow")
        nc.sync.dma_start(vnrow[:], row_of(agon, 1))

        # ---------------- final assembly ----------------
        # S_core = dotZ/n - sum(K.B) - P1/n + mbL*P2 + mbK*P3 - 512*n*mbK*mbL
        # where B = q_j - L, dotZ = sum_j R^K_j * colB_j (colB over local rows),
        # P1 = sum_i R^K_i R^L_i, P2 = sum_i R^K_i, P3 = sum_i R^L_i (local i),
        # mbK/mbL the grand means of K/L.
        # sum(K.B): one fused pass, per-partition accum
        kb4 = small.tile([128, MT], f32, tag="kb4", name="kb4")
        for m in range(MT):
            nc.vector.scalar_tensor_tensor(
                scr16[:], d2sz[:, m, :], 1.0, d2sn[:, m, :], Alu.mult, Alu.mult,
                accum_out=kb4[:, m:m + 1])

        # colB via ones-matmuls on PE (B is fp16)
        ones1h = small.tile([128, 1], f16, tag="ones1h", name="ones1h")
        nc.vector.memset(ones1h[:], 1.0)
        colb = small.tile([1, NTOT], f32, tag="colb", name="colb")
        for h in range(2):
            pc = psum.tile([1, 2048], f32, tag="ps", name=f"pcolb{h}")
            for q in range(4):
                cslice = slice(h * 2048 + q * 512, h * 2048 + (q + 1) * 512)
                for m in range(MT):
                    nc.tensor.matmul(pc[:, q * 512:(q + 1) * 512], ones1h[:],
                                     d2sn[:, m, cslice],
                                     start=(m == 0), stop=(m == MT - 1))
            nc.scalar.activation(colb[:, h * 2048:(h + 1) * 2048], pc[:],
                                 Act.Identity)

        # per-partition pieces -> one [1,4] partition-sum matmul
        u1 = small.tile([128, 1], f32, tag="u1", name="u1")
        nc.vector.scalar_tensor_tensor(scr16[:, 0:MT], rz[:], 1.0, rn[:],
                                       Alu.mult, Alu.mult, accum_out=u1[:, 0:1])
        u2 = small.tile([128, 1], f32, tag="u2", name="u2")
        nc.vector.tensor_reduce(u2[:], rz[:], mybir.AxisListType.X, Alu.add)
        u3 = small.tile([128, 1], f32, tag="u3", name="u3")
        nc.vector.tensor_reduce(u3[:], rn[:], mybir.AxisListType.X, Alu.add)
        wq = small.tile([128, 4], f32, tag="wq", name="wq")
        nc.vector.tensor_copy(wq[:, 0:1], u1[:])
        nc.vector.tensor_copy(wq[:, 1:2], u2[:])
        nc.vector.tensor_copy(wq[:, 2:3], u3[:])
        nc.vector.tensor_reduce(wq[:, 3:4], kb4[:], mybir.AxisListType.X, Alu.add)
        wp = psum.tile([1, 4], f32, tag="ps", name="wp")
        nc.tensor.matmul(wp[:], ones1[:], wq[:], start=True, stop=True)
        ws = small.tile([1, 4], f32, tag="ws", name="ws")
        nc.scalar.activation(ws[:], wp[:], Act.Identity)

        # dotZ = sum_j R^K_j * colB_j  (single-partition fused pass)
        scrow = small.tile([1, NTOT], f16, tag="scrow", name="scrow")
        dz1 = small.tile([1, 1], f32, tag="dz1", name="dz1")
        nc.vector.scalar_tensor_tensor(scrow[:], colb[:], inv_n, vzrow[:],
                                       Alu.mult, Alu.mult, accum_out=dz1[:, 0:1])
        # grand means
        tkr = small.tile([1, 1], f32, tag="tkr", name="tkr")
        nc.scalar.activation(scrow[:], vzrow[:], Act.Identity, accum_out=tkr[:, 0:1])
        tlr = small.tile([1, 1], f32, tag="tlr", name="tlr")
        nc.scalar.activation(scrow[:], vnrow[:], Act.Identity, accum_out=tlr[:, 0:1])
        mbk = small.tile([1, 1], f32, tag="mbk", name="mbk")
        nc.vector.tensor_scalar(mbk[:], tkr[:], inv_n2, None, Alu.mult)
        mbl = small.tile([1, 1], f32, tag="mbl", name="mbl")
        nc.vector.tensor_scalar(mbl[:], tlr[:], inv_n2, None, Alu.mult)

        # combine: S = dz1 - P1/n - KB + mbl*P2 + mbk*P3 - 512*n*mbk*mbl
        e1 = small.tile([1, 1], f32, tag="e1", name="e1")
        nc.vector.tensor_scalar(e1[:], ws[0:1, 0:1], inv_n, dz1[:],
                                Alu.mult, Alu.subtract)      # P1/n - dz1
        e2 = small.tile([1, 1], f32, tag="e2", name="e2")
        nc.vector.tensor_add(e2[:], e1[:], ws[0:1, 3:4])     # P1/n - dz1 + KB
        e3 = small.tile([1, 1], f32, tag="e3", name="e3")
        nc.vector.tensor_mul(e3[:], mbl[:], ws[0:1, 1:2])    # mbl*P2
        e4 = small.tile([1, 1], f32, tag="e4", name="e4")
        nc.vector.tensor_mul(e4[:], mbk[:], ws[0:1, 2:3])    # mbk*P3
        e5 = small.tile([1, 1], f32, tag="e5", name="e5")
        nc.vector.tensor_mul(e5[:], mbk[:], mbl[:])
        e6 = small.tile([1, 1], f32, tag="e6", name="e6")
        nc.vector.tensor_scalar(e6[:], e5[:], -float(BLK * NTOT), None, Alu.mult)
        e7 = small.tile([1, 1], f32, tag="e7", name="e7")
        nc.vector.tensor_add(e7[:], e3[:], e4[:])
        e8 = small.tile([1, 1], f32, tag="e8", name="e8")
        nc.vector.tensor_add(e8[:], e7[:], e6[:])
        sfin = small.tile([1, 1], f32, tag="sfin", name="sfin")
        nc.vector.tensor_sub(sfin[:], e8[:], e2[:])
        nc.sync.dma_start(out_s[:], sfin[0:1, 0:1])

        # debug outputs
        nc.sync.dma_start(out_dbg[0:1, 0:1], medz[0:1, 0:1])
        nc.sync.dma_start(out_dbg[0:1, 1:2], medn[0:1, 0:1])
        nc.sync.dma_start(out_dbg[0:1, 2:4], cgz[0:1, :])
        nc.sync.dma_start(out_dbg[0:1, 4:6], cgn[0:1, :])
        nc.sync.dma_start(out_dbg[0:1, 6:7], tkr[0:1, 0:1])
        nc.sync.dma_start(out_dbg[0:1, 7:8], tlr[0:1, 0:1])

    return nc


def _get_nc():
    if "nc" not in _nc_cache:
        nc = _build()
        _split_waits(nc)
        _nc_cache["nc"] = nc
    return _nc_cache["nc"]


def _sample_median(X32, xsq):
    """Host estimate of the lower-median of the pairwise squared distances."""
    rows = X32[::8]
    cols = X32[::2]
    G = rows @ cols.T
    d2 = xsq[::8, None] + xsq[None, ::2] - 2.0 * G
    flat = d2.ravel()
    return float(np.partition(flat, (flat.size - 1) // 2)[(flat.size - 1) // 2])


def _prepare_inputs(Z, N):
    Zf = np.asarray(Z, dtype=np.float32)
    Nf = np.asarray(N, dtype=np.float32)
    zsq = (Zf.astype(np.float64) ** 2).sum(1).astype(np.float32)
    nsq = (Nf.astype(np.float64) ** 2).sum(1).astype(np.float32)
    Zb = Zf.astype(_BF16)
    Nb = Nf.astype(_BF16)

    def aug(Xb, xsq):
        w = (-0.5 * xsq).astype(np.float32)
        w_hi = w.astype(_BF16)
        w_lo = (w - w_hi.astype(np.float32)).astype(_BF16)
        return np.concatenate(
            [np.ascontiguousarray(Xb.T), w_hi[None, :], w_lo[None, :]], axis=0)

    zt = aug(Zb, zsq)
    nt = aug(Nb, nsq)

    t0z = _sample_median(Zf, zsq)
    t0n = _sample_median(Nf, nsq)
    thr = np.array([t0z - HZ - SH_Z, t0z + HZ - SH_Z,
                    t0n - HN - SH_N, t0n + HN - SH_N], dtype=np.float32)
    # keep thresholds off the fp16 grid so is_le sees no exact ties
    on_grid = thr == thr.astype(np.float16).astype(np.float32)
    thr[on_grid] += np.float32(1.001953125e-3)

    in_maps = []
    for c in range(NCORES):
        sl = slice(c * BLK, (c + 1) * BLK)
        in_maps.append({
            "zt": zt,
            "ntr": nt,
            "lhsz": np.ascontiguousarray(Zb.T[:, sl]),
            "lhsn": np.ascontiguousarray(Nb.T[:, sl]),
            "zsqm": (zsq[sl] - SH_Z).astype(np.float32),
            "nsqm": (nsq[sl] - SH_N).astype(np.float32),
            "thr": thr,
        })
    return in_maps


def run_on_device(Z, N, **run_kwargs):
    """Run the bass kernel; returns (BassKernelResults, hsic float)."""
    from concourse.bass_utils import run_bass_kernel_spmd
    nc = _get_nc()
    in_maps = _prepare_inputs(Z, N)
    res = run_bass_kernel_spmd(nc, in_maps, core_ids=list(range(NCORES)),
                               **run_kwargs)
    S = sum(float(r["out_s"][0, 0]) for r in res.results)
    hsic = S / ((NTOT - 1) ** 2 + 1e-8)
    return res, hsic


def kernel(Z, N):
    _, hsic = run_on_device(Z, N)
    return np.asarray(hsic, dtype=np.float32)


if __name__ == "__main__":
    rng = np.random.default_rng(0)
    Z = rng.standard_normal((NTOT, DZ), dtype=np.float32)
    N = rng.standard_normal((NTOT, DN), dtype=np.float32)
    res, hsic = run_on_device(Z, N)
    print("hsic:", hsic)
    print("dbg core0:", res.results[0]["out_dbg"])
